# revision 1
# baseline (speedup 1.0000x reference)
"""Trainium2 Bass kernel for nn_AttnAdapter: GQA attention with RoPE,
region-based enhance/suppress score scaling, causal mask, o_proj.

Sharding: tensor-parallel over heads across 8 NeuronCores. Core d holds
q-heads 4d..4d+3 (wq rows), kv-head d (wk/wv rows), and wo columns
512d..512(d+1). Each core computes a full [S, D] partial of the output;
the host sums the 8 partials (the TP all-reduce, done at unshard time).

All on-chip matmuls run in float32r (full PE rate for N>=512) over
transposed layouts so no on-device transposes are needed except V
(16 PE 128x128 transposes). xT and the output are host-tiled so every
DMA moves one fully contiguous 256KB block.
"""

import math

import numpy as np

# ---- problem constants (hardcoded; kernel.py must be self-contained) ----
S = 2048          # sequence length
D = 4096          # model dim
HD = 128          # head dim
NCORES = 8
QH = 4            # q heads per core
SYS_LEN, IMG_LEN = 35, 576
BOUND = SYS_LEN + IMG_LEN          # 611
ENH, SUP = 1.5, 0.5
ROPE_BASE = 10000.0

J = 4             # sq tiles of 512
NSK = 16          # sk tiles of 128
DCH = 32          # D chunks of 128

_CACHE = {}


def _host_constants():
    inv_freq = 1.0 / (ROPE_BASE ** (np.arange(0, HD, 2, dtype=np.float32) / HD))
    pos = np.arange(S, dtype=np.float32)
    freqs = pos[:, None] * inv_freq[None, :]              # [S, 64]
    emb = np.concatenate([freqs, freqs], axis=-1)         # [S, 128]
    cosT = np.ascontiguousarray(np.cos(emb).T.astype(np.float32))  # [128, S]
    sinT = np.ascontiguousarray(np.sin(emb).T.astype(np.float32))

    # rotate_half as a matmul: rot = R @ q (in [hd, s] layout).
    # matmul(out, lhsT, rhs) = lhsT.T @ rhs, so feed RT = R.T.
    RT = np.zeros((HD, HD), dtype=np.float32)
    half = HD // 2
    for c in range(half):
        RT[c + half, c] = -1.0      # rot[c] = -q[c+64]
    for c in range(half, HD):
        RT[c - half, c] = 1.0       # rot[c] = q[c-64]

    ident = np.eye(HD, dtype=np.float32)

    # Diagonal-tile causal masks, T layout [sk 128, sq 512]:
    # tile (i=4j+delta, j): valid (keep) iff sq >= sk  <=>  f >= 128*delta + p
    masks = np.zeros((HD, 4 * 512), dtype=np.float32)
    p = np.arange(128)[:, None]
    f = np.arange(512)[None, :]
    for delta in range(4):
        masks[:, delta * 512:(delta + 1) * 512] = (f >= 128 * delta + p)

    # key_scale in partition layout per sk-tile: ksT[p, i] = scale(128*i+p)
    kpos = np.arange(S)
    key_scale = np.where(kpos < SYS_LEN, SUP,
                         np.where(kpos < BOUND, ENH, 1.0)).astype(np.float32)
    ksT = np.ascontiguousarray(key_scale[:5 * 128].reshape(5, 128).T)  # [128, 5]

    ones_col = np.ones((HD, 1), dtype=np.float32)
    return dict(cosT=cosT, sinT=sinT, rmat=RT, ident=ident, masks=masks,
                ksT=ksT, ones_col=ones_col)


def _build_bass():
    import concourse.bass as bass
    import concourse.mybir as mybir
    from concourse.tile import TileContext
    from contextlib import ExitStack

    f32 = mybir.dt.float32
    f32r = mybir.dt.float32r

    nc = bass.Bass()
    # xTt[d, j, p, f] = x.T[128d+p, 512j+f] -- each (d,j) tile contiguous
    xTt = nc.dram_tensor("xTt", [DCH, J, 128, 512], f32r, kind="ExternalInput")
    wqT = nc.dram_tensor("wqT", [D, QH * HD], f32r, kind="ExternalInput")
    wkT = nc.dram_tensor("wkT", [D, HD], f32r, kind="ExternalInput")
    wvT = nc.dram_tensor("wvT", [D, HD], f32r, kind="ExternalInput")
    woT = nc.dram_tensor("woT", [QH * HD, D], f32r, kind="ExternalInput")
    cosT_d = nc.dram_tensor("cosT", [HD, S], f32, kind="ExternalInput")
    sinT_d = nc.dram_tensor("sinT", [HD, S], f32, kind="ExternalInput")
    rmat_d = nc.dram_tensor("rmat", [HD, HD], f32r, kind="ExternalInput")
    ident_d = nc.dram_tensor("ident", [HD, HD], f32, kind="ExternalInput")
    masks_d = nc.dram_tensor("masks", [HD, 4 * 512], f32, kind="ExternalInput")
    ksT_d = nc.dram_tensor("ksT", [HD, 5], f32, kind="ExternalInput")
    onesc_d = nc.dram_tensor("ones_col", [HD, 1], f32r, kind="ExternalInput")
    onesr_d = nc.dram_tensor("ones_row", [1, HD], f32r, kind="ExternalInput")
    # out_t[t, n, p, f] = out[128t+p, 512n+f] -- contiguous per tile
    out = nc.dram_tensor("out", [NSK, 8, 128, 512], f32, kind="ExternalOutput")

    EXP = mybir.ActivationFunctionType.Exp

    with TileContext(nc) as tc, ExitStack() as ctx:
        const = ctx.enter_context(tc.tile_pool(name="const", bufs=1))
        cosT = const.tile([HD, S], f32)
        nc.sync.dma_start(cosT[:], cosT_d[:, :])
        sinT = const.tile([HD, S], f32)
        nc.sync.dma_start(sinT[:], sinT_d[:, :])
        rmat = const.tile([HD, HD], f32r)
        nc.sync.dma_start(rmat[:], rmat_d[:, :])
        ident = const.tile([HD, HD], f32)
        nc.sync.dma_start(ident[:], ident_d[:, :])
        masks = const.tile([HD, 4 * 512], f32)
        nc.sync.dma_start(masks[:], masks_d[:, :])
        ksT = const.tile([HD, 5], f32)
        nc.sync.dma_start(ksT[:], ksT_d[:, :])
        ones_col = const.tile([HD, 1], f32r)
        nc.sync.dma_start(ones_col[:], onesc_d[:, :])
        ones_row = const.tile([1, HD], f32r)
        nc.sync.dma_start(ones_row[:], onesr_d[:, :])

        persist = ctx.enter_context(tc.tile_pool(name="persist", bufs=1))
        qrot = [persist.tile([HD, S], f32r, name=f"qrot{m}") for m in range(QH)]
        krot = persist.tile([HD, S], f32r)
        vnat = persist.tile([HD, NSK * HD], f32r)  # tile i at cols i*128
        attn = [persist.tile([HD, S], f32r, name=f"attn{h}") for h in range(QH)]

        # ---------------- Phase A: projections + RoPE + V transpose --------
        with tc.tile_pool(name="xw", bufs=4) as xw, \
             tc.tile_pool(name="accp", bufs=1, space="PSUM") as accp, \
             tc.tile_pool(name="ropep", bufs=2, space="PSUM") as ropep, \
             tc.tile_pool(name="stage", bufs=3) as stage:
            for j in range(J):
                sq = slice(j * 512, (j + 1) * 512)
                accs = [accp.tile([128, 512], f32, name=f"acc{m}") for m in range(6)]
                for d in range(DCH):
                    dd = slice(d * 128, (d + 1) * 128)
                    xt = xw.tile([128, 512], f32r, tag="xt")
                    nc.sync.dma_start(xt[:], xTt[d, j])
                    wq_t = xw.tile([128, 512], f32r, tag="wq")
                    nc.sync.dma_start(wq_t[:], wqT[dd, :])
                    wkv_t = xw.tile([128, 256], f32r, tag="wkv")
                    nc.sync.dma_start(wkv_t[:, 0:128], wkT[dd, :])
                    nc.sync.dma_start(wkv_t[:, 128:256], wvT[dd, :])
                    st = (d == 0)
                    sp = (d == DCH - 1)
                    for m in range(QH):
                        nc.tensor.matmul(accs[m][:], wq_t[:, m * 128:(m + 1) * 128],
                                         xt[:], start=st, stop=sp)
                    nc.tensor.matmul(accs[4][:], wkv_t[:, 0:128], xt[:],
                                     start=st, stop=sp)
                    nc.tensor.matmul(accs[5][:], wkv_t[:, 128:256], xt[:],
                                     start=st, stop=sp)

                # RoPE for q tiles and k tile; PSUM released by the ACT copy
                for m in range(5):
                    dst = qrot[m][:, sq] if m < QH else krot[:, sq]
                    q_sb = stage.tile([128, 512], f32r, tag="q_sb")
                    nc.scalar.copy(q_sb[:], accs[m][:])
                    rot_ps = ropep.tile([128, 512], f32, tag="rope_ps")
                    nc.tensor.matmul(rot_ps[:], rmat[:], q_sb[:],
                                     start=True, stop=True)
                    t1 = stage.tile([128, 512], f32, tag="t1")
                    nc.vector.tensor_mul(t1[:], q_sb[:], cosT[:, sq])
                    t2 = stage.tile([128, 512], f32, tag="t2")
                    nc.vector.tensor_mul(t2[:], rot_ps[:], sinT[:, sq])
                    nc.vector.tensor_add(dst, t1[:], t2[:])

                # V: copy to SBUF, transpose 128x128 blocks into vnat
                v_sb = stage.tile([128, 512], f32, tag="v_sb")
                nc.scalar.copy(v_sb[:], accs[5][:])
                for b in range(4):
                    i = 4 * j + b
                    vt_ps = ropep.tile([128, 512], f32, tag="rope_ps")
                    nc.tensor.transpose(vt_ps[:, 0:128],
                                        v_sb[:, b * 128:(b + 1) * 128], ident[:])
                    nc.vector.tensor_copy(vnat[:, i * 128:(i + 1) * 128],
                                          vt_ps[:, 0:128])

        # woT loads issued here so they prefetch during phase B
        wo_sb = ctx.enter_context(tc.tile_pool(name="wo_sb", bufs=1))
        wo_t = [wo_sb.tile([128, D], f32r, name=f"wo{h}") for h in range(QH)]
        for h in range(QH):
            nc.sync.dma_start(wo_t[h][:], woT[h * 128:(h + 1) * 128, :])

        # ---------------- Phase B: attention ------------------------------
        with tc.tile_pool(name="att_sb", bufs=4) as att_sb, \
             tc.tile_pool(name="sp", bufs=2, space="PSUM") as sp, \
             tc.tile_pool(name="avp", bufs=2, space="PSUM") as avp, \
             tc.tile_pool(name="dnp", bufs=2, space="PSUM") as dnp, \
             tc.tile_pool(name="rbp", bufs=2, space="PSUM") as rbp, \
             tc.tile_pool(name="nrm", bufs=3) as nrm:
            for j in range(J):
                sq = slice(j * 512, (j + 1) * 512)
                ni = 4 * j + 4            # sk tiles 0..4j+3 are live
                for h in range(QH):
                    acc_av = avp.tile([128, 512], f32, tag="av")
                    acc_dn = dnp.tile([1, 512], f32, tag="dn")
                    for i in range(ni):
                        s_ps = sp.tile([128, 512], f32, tag="s")
                        nc.tensor.matmul(s_ps[:], krot[:, i * 128:(i + 1) * 128],
                                         qrot[h][:, sq], start=True, stop=True)
                        if i < 5:
                            # region enhance/suppress for sq >= BOUND
                            c0 = 0 if j >= 2 else (BOUND - 512 if j == 1 else None)
                            if c0 is not None:
                                nc.vector.tensor_scalar_mul(
                                    s_ps[:, c0:512], s_ps[:, c0:512],
                                    ksT[:, i:i + 1])
                        e_sb = att_sb.tile([128, 512], f32r, tag="e")
                        nc.scalar.activation(e_sb[:], s_ps[:], EXP)
                        delta = i - 4 * j
                        if delta >= 0:
                            nc.vector.tensor_mul(
                                e_sb[:], e_sb[:],
                                masks[:, delta * 512:(delta + 1) * 512])
                        st = (i == 0)
                        sp_l = (i == ni - 1)
                        nc.tensor.matmul(acc_dn[:], ones_col[:], e_sb[:],
                                         start=st, stop=sp_l)
                        nc.tensor.matmul(acc_av[:], vnat[:, i * 128:(i + 1) * 128],
                                         e_sb[:], start=st, stop=sp_l)
                    # softmax denom -> 1/x = exp(-ln(x)) on ACT -> K=1 bcast
                    lrec = nrm.tile([1, 512], f32, tag="lrec")
                    nc.scalar.activation(lrec[:], acc_dn[:],
                                         mybir.ActivationFunctionType.Ln)
                    rec2 = nrm.tile([1, 512], f32r, tag="rec2")
                    nc.scalar.activation(rec2[:], lrec[:], EXP, scale=-1.0)
                    rb_ps = rbp.tile([128, 512], f32, tag="rb")
                    nc.tensor.matmul(rb_ps[:], ones_row[:], rec2[:],
                                     start=True, stop=True)
                    rb_sb = nrm.tile([128, 512], f32, tag="rb_sb")
                    nc.scalar.copy(rb_sb[:], rb_ps[:])
                    nc.vector.tensor_mul(attn[h][:, sq], acc_av[:], rb_sb[:])

        # ---------------- Phase C: o_proj ---------------------------------
        with tc.tile_pool(name="op", bufs=4, space="PSUM") as op, \
             tc.tile_pool(name="ost", bufs=4) as ost:
            for t in range(NSK):
                ts_ = slice(t * 128, (t + 1) * 128)
                for n in range(8):
                    o_ps = op.tile([128, 512], f32, tag="o")
                    for h in range(QH):
                        nc.tensor.matmul(o_ps[:], attn[h][:, ts_],
                                         wo_t[h][:, n * 512:(n + 1) * 512],
                                         start=(h == 0), stop=(h == QH - 1))
                    o_sb = ost.tile([128, 512], f32, tag="o_sb")
                    nc.any.tensor_copy(o_sb[:], o_ps[:])
                    nc.sync.dma_start(out[t, n], o_sb[:])

    # Split multi-wait instructions (self-loading f32r matmuls allow only
    # one sync wait) onto standalone EventSemaphore instructions.
    import bass_rust
    bass_rust.generate_event_semaphores(nc)
    return nc


def _get_compiled():
    if "nc" not in _CACHE:
        _CACHE["nc"] = _build_bass()
        _CACHE["const"] = _host_constants()
    return _CACHE["nc"], _CACHE["const"]


def kernel(hidden_states, wq, wk, wv, wo, _trace=False):
    from concourse.bass_utils import run_bass_kernel_spmd

    nc, cst = _get_compiled()

    x = np.asarray(hidden_states, dtype=np.float32).reshape(S, D)
    xT = np.ascontiguousarray(x.T)                       # [D, S]
    xTt = np.ascontiguousarray(
        xT.reshape(DCH, 128, J, 512).transpose(0, 2, 1, 3))  # [DCH,J,128,512]
    wq = np.asarray(wq, dtype=np.float32)
    wk = np.asarray(wk, dtype=np.float32)
    wv = np.asarray(wv, dtype=np.float32)
    wo = np.asarray(wo, dtype=np.float32)
    scale = 1.0 / math.sqrt(HD)

    in_maps = []
    for d in range(NCORES):
        wq_d = wq[d * QH * HD:(d + 1) * QH * HD] * scale      # [512, D]
        in_maps.append({
            "xTt": xTt,
            "wqT": np.ascontiguousarray(wq_d.T),
            "wkT": np.ascontiguousarray(wk[d * HD:(d + 1) * HD].T),
            "wvT": np.ascontiguousarray(wv[d * HD:(d + 1) * HD].T),
            "woT": np.ascontiguousarray(wo[:, d * QH * HD:(d + 1) * QH * HD].T),
            "cosT": cst["cosT"], "sinT": cst["sinT"],
            "rmat": cst["rmat"], "ident": cst["ident"],
            "masks": cst["masks"], "ksT": cst["ksT"],
            "ones_col": cst["ones_col"],
            "ones_row": np.ones((1, HD), dtype=np.float32),
        })

    res = run_bass_kernel_spmd(nc, in_maps, core_ids=list(range(NCORES)),
                               trace=_trace)
    acc = res.results[0]["out"].astype(np.float64)
    for d in range(1, NCORES):
        acc += res.results[d]["out"]
    # out_t[t, n, p, f] -> out[128t+p, 512n+f]
    outp = acc.transpose(0, 2, 1, 3).reshape(S, D).astype(np.float32)
    outp = outp.reshape(1, S, D)
    if _trace:
        _CACHE["last_results"] = res
    return outp



# revision 6
# speedup vs baseline: 1.2598x; 1.2598x over previous
"""Trainium2 Bass kernel for nn_AttnAdapter: GQA attention with RoPE,
region-based enhance/suppress score scaling, causal mask, o_proj.

Sharding: tensor-parallel over heads across 8 NeuronCores. Core d holds
q-heads 4d..4d+3 (wq rows), kv-head d (wk/wv rows), and wo columns
512d..512(d+1). Each core computes a full [S, D] partial of the output;
the host sums the 8 partials (the TP all-reduce, done at unshard time).

v2: all matmuls in bf16 (fp32 PSUM accumulate), weights resident in
SBUF (single load), exp batched over [128,1024] PSUM pairs, region
scale folded into pre-scaled K columns, reciprocal on DVE, all PSUM
evictions on DVE, batched DMAs.
"""

import math

import numpy as np

# ---- problem constants (hardcoded; kernel.py must be self-contained) ----
S = 2048          # sequence length
D = 4096          # model dim
HD = 128          # head dim
NCORES = 8
QH = 4            # q heads per core
SYS_LEN, IMG_LEN = 35, 576
BOUND = SYS_LEN + IMG_LEN          # 611
ENH, SUP = 1.5, 0.5
ROPE_BASE = 10000.0

J = 4             # sq tiles of 512
NSK = 16          # sk tiles of 128
DCH = 32          # D chunks of 128
G = 4             # d-groups of 8 chunks
GD = 8            # d-chunks per group

_CACHE = {}


def _bf16():
    import ml_dtypes
    return ml_dtypes.bfloat16


def _host_constants():
    bf16 = _bf16()
    inv_freq = 1.0 / (ROPE_BASE ** (np.arange(0, HD, 2, dtype=np.float32) / HD))
    pos = np.arange(S, dtype=np.float32)
    freqs = pos[:, None] * inv_freq[None, :]              # [S, 64]
    emb = np.concatenate([freqs, freqs], axis=-1)         # [S, 128]
    cosT = np.ascontiguousarray(np.cos(emb).T.astype(np.float32))  # [128, S]
    sinT = np.ascontiguousarray(np.sin(emb).T.astype(np.float32))

    # rotate_half as a matmul: rot = R @ q (in [hd, s] layout).
    # matmul(out, lhsT, rhs) = lhsT.T @ rhs, so feed RT = R.T.
    RT = np.zeros((HD, HD), dtype=np.float32)
    half = HD // 2
    for c in range(half):
        RT[c + half, c] = -1.0      # rot[c] = -q[c+64]
    for c in range(half, HD):
        RT[c - half, c] = 1.0       # rot[c] = q[c-64]

    ident = np.eye(HD, dtype=np.float32)

    # Diagonal-tile causal masks, T layout [sk 128, sq 512]:
    # tile (i=4j+delta, j): valid (keep) iff sq >= sk  <=>  f >= 128*delta + p
    masks = np.zeros((HD, 4 * 512), dtype=np.float32)
    p = np.arange(128)[:, None]
    f = np.arange(512)[None, :]
    for delta in range(4):
        masks[:, delta * 512:(delta + 1) * 512] = (f >= 128 * delta + p)

    kpos = np.arange(S)
    key_scale = np.where(kpos < SYS_LEN, SUP,
                         np.where(kpos < BOUND, ENH, 1.0)).astype(np.float32)
    # per-partition scale per sk-tile (j=1 partial-row DVE path)
    ksT = np.ascontiguousarray(key_scale[:5 * 128].reshape(5, 128).T)  # [128, 5]
    # per-key-column broadcast (for pre-scaling krot columns, j>=2 path)
    ksB = np.ascontiguousarray(
        np.broadcast_to(key_scale[:640][None, :], (HD, 640)))          # [128, 640]

    return dict(
        cosT=cosT, sinT=sinT,
        rmat=RT.astype(bf16), ident=ident.astype(bf16),
        masks=masks.astype(bf16), ksT=ksT, ksB=ksB.astype(bf16),
        ones_col=np.ones((HD, 1), dtype=bf16),
        ones_row=np.ones((1, HD), dtype=np.float32),
    )


def _build_bass():
    import concourse.bass as bass
    import concourse.mybir as mybir
    from concourse.tile import TileContext
    from contextlib import ExitStack

    f32 = mybir.dt.float32
    f32r = mybir.dt.float32r
    bf = mybir.dt.bfloat16

    nc = bass.Bass()
    # x packed per (j, g): [128, GD*512] with [p, dd*512+f] = x.T[128(8g+dd)+p, 512j+f]
    xp = nc.dram_tensor("xp", [J, G, 128, GD * 512], bf, kind="ExternalInput")
    # wq packed per g: [128, GD*512] with [p, dd*512+f] = wq_d.T[128(8g+dd)+p, f]
    wqp = nc.dram_tensor("wqp", [G, 128, GD * 512], bf, kind="ExternalInput")
    # wk/wv packed per g: [128, GD*256], cols dd*256+0:128 = wkT, 128:256 = wvT
    wkvp = nc.dram_tensor("wkvp", [G, 128, GD * 256], bf, kind="ExternalInput")
    # wo packed: [128, 4*4096] with [p, h*4096+f] = wo[:, core].T[128h+p, f]
    wop = nc.dram_tensor("wop", [128, QH * D], bf, kind="ExternalInput")
    cosT_d = nc.dram_tensor("cosT", [HD, S], f32, kind="ExternalInput")
    sinT_d = nc.dram_tensor("sinT", [HD, S], f32, kind="ExternalInput")
    rmat_d = nc.dram_tensor("rmat", [HD, HD], bf, kind="ExternalInput")
    ident_d = nc.dram_tensor("ident", [HD, HD], bf, kind="ExternalInput")
    masks_d = nc.dram_tensor("masks", [HD, 4 * 512], bf, kind="ExternalInput")
    ksT_d = nc.dram_tensor("ksT", [HD, 5], f32, kind="ExternalInput")
    ksB_d = nc.dram_tensor("ksB", [HD, 640], bf, kind="ExternalInput")
    onesc_d = nc.dram_tensor("ones_col", [HD, 1], bf, kind="ExternalInput")
    onesr_d = nc.dram_tensor("ones_row", [1, HD], f32r, kind="ExternalInput")
    # out.T tiles: out_t[j, cq, p, cb*512+f] = out.T[128*(4cq+cb)+p, 512j+f]
    out = nc.dram_tensor("out", [J, DCH // 4, 128, 4 * 512], bf,
                         kind="ExternalOutput")

    EXP = mybir.ActivationFunctionType.Exp

    with TileContext(nc) as tc, ExitStack() as ctx:
        # ---- resident weights (single load, used all phases) ----
        wsb = ctx.enter_context(tc.tile_pool(name="wsb", bufs=1))
        wq_sb = wsb.tile([128, DCH * 512], bf)
        wkv_sb = wsb.tile([128, DCH * 256], bf)
        for g in range(G):
            nc.sync.dma_start(wq_sb[:, g * GD * 512:(g + 1) * GD * 512], wqp[g])
            nc.sync.dma_start(wkv_sb[:, g * GD * 256:(g + 1) * GD * 256], wkvp[g])

        const = ctx.enter_context(tc.tile_pool(name="const", bufs=1))
        cosT = const.tile([HD, S], f32)
        nc.sync.dma_start(cosT[:], cosT_d[:, :])
        sinT = const.tile([HD, S], f32)
        nc.sync.dma_start(sinT[:], sinT_d[:, :])
        rmat = const.tile([HD, HD], bf)
        nc.sync.dma_start(rmat[:], rmat_d[:, :])
        ident = const.tile([HD, HD], bf)
        nc.sync.dma_start(ident[:], ident_d[:, :])
        masks = const.tile([HD, 4 * 512], bf)
        nc.sync.dma_start(masks[:], masks_d[:, :])
        ksT = const.tile([HD, 5], f32)
        nc.sync.dma_start(ksT[:], ksT_d[:, :])
        ksB = const.tile([HD, 640], bf)
        nc.sync.dma_start(ksB[:], ksB_d[:, :])
        ones_col = const.tile([HD, 1], bf)
        nc.sync.dma_start(ones_col[:], onesc_d[:, :])
        ones_row = const.tile([1, HD], f32r)
        nc.sync.dma_start(ones_row[:], onesr_d[:, :])
        wo_sb = const.tile([128, QH * D], bf)
        nc.sync.dma_start(wo_sb[:], wop[:, :])

        persist = ctx.enter_context(tc.tile_pool(name="persist", bufs=1))
        qrot = [persist.tile([HD, S], bf, name=f"qrot{m}") for m in range(QH)]
        krot = persist.tile([HD, S], bf)
        krot_s = persist.tile([HD, 640], bf)     # region-scaled K cols (keys<640)
        vnat = persist.tile([HD, NSK * HD], bf)  # tile i at cols i*128
        attn = [persist.tile([HD, S], bf, name=f"attn{h}") for h in range(QH)]

        # ---------------- Phase A: projections + RoPE + V transpose --------
        with tc.tile_pool(name="xg", bufs=2) as xgp, \
             tc.tile_pool(name="accp", bufs=1, space="PSUM") as accp, \
             tc.tile_pool(name="ropep", bufs=1, space="PSUM") as ropep, \
             tc.tile_pool(name="vtp", bufs=1, space="PSUM") as vtp, \
             tc.tile_pool(name="stage", bufs=2) as stage:
            for j in range(J):
                sq = slice(j * 512, (j + 1) * 512)
                accs = [accp.tile([128, 512], f32, name=f"acc{m}") for m in range(6)]
                for g in range(G):
                    xt = xgp.tile([128, GD * 512], bf, tag="xt")
                    nc.sync.dma_start(xt[:], xp[j, g])
                    for dd in range(GD):
                        d = g * GD + dd
                        xs = xt[:, dd * 512:(dd + 1) * 512]
                        st = (d == 0)
                        sp_ = (d == DCH - 1)
                        qb = d * 512
                        for m in range(QH):
                            nc.tensor.matmul(
                                accs[m][:], wq_sb[:, qb + m * 128:qb + (m + 1) * 128],
                                xs, start=st, stop=sp_)
                        kb = d * 256
                        nc.tensor.matmul(accs[4][:], wkv_sb[:, kb:kb + 128],
                                         xs, start=st, stop=sp_)
                        nc.tensor.matmul(accs[5][:], wkv_sb[:, kb + 128:kb + 256],
                                         xs, start=st, stop=sp_)

                # RoPE for q tiles and k tile
                for m in range(5):
                    dst = qrot[m][:, sq] if m < QH else krot[:, sq]
                    q_sb = stage.tile([128, 512], bf, tag="q_sb")
                    nc.scalar.copy(q_sb[:], accs[m][:])
                    rot_ps = ropep.tile([128, 512], f32, tag="rope_ps")
                    nc.tensor.matmul(rot_ps[:], rmat[:], q_sb[:],
                                     start=True, stop=True)
                    t1 = stage.tile([128, 512], f32, tag="t1")
                    nc.vector.tensor_mul(t1[:], accs[m][:], cosT[:, sq])
                    t2 = stage.tile([128, 512], f32, tag="t2")
                    nc.vector.tensor_mul(t2[:], rot_ps[:], sinT[:, sq])
                    nc.vector.tensor_add(dst, t1[:], t2[:])
                # region-scaled K columns (keys < 640 only)
                if j == 0:
                    nc.vector.tensor_mul(krot_s[:, 0:512], krot[:, 0:512],
                                         ksB[:, 0:512])
                elif j == 1:
                    nc.vector.tensor_mul(krot_s[:, 512:640], krot[:, 512:640],
                                         ksB[:, 512:640])

                # V: copy to SBUF, transpose 128x128 blocks into vnat
                v_sb = stage.tile([128, 512], bf, tag="v_sb")
                nc.scalar.copy(v_sb[:], accs[5][:])
                vt_ps = vtp.tile([128, 512], bf, tag="vt_ps")
                for b in range(4):
                    nc.tensor.transpose(vt_ps[:, b * 128:(b + 1) * 128],
                                        v_sb[:, b * 128:(b + 1) * 128], ident[:])
                nc.vector.tensor_copy(
                    vnat[:, (4 * j) * 128:(4 * j + 4) * 128], vt_ps[:])

        # ---------------- Phase B: attention ------------------------------
        with tc.tile_pool(name="att_sb", bufs=3) as att_sb, \
             tc.tile_pool(name="sp", bufs=2, space="PSUM") as sp, \
             tc.tile_pool(name="avp", bufs=2, space="PSUM") as avp, \
             tc.tile_pool(name="dnp", bufs=1, space="PSUM") as dnp, \
             tc.tile_pool(name="rbp", bufs=1, space="PSUM") as rbp, \
             tc.tile_pool(name="nrm", bufs=2) as nrm:
            for j in range(J):
                sq = slice(j * 512, (j + 1) * 512)
                ni = 4 * j + 4            # sk tiles 0..4j+3 are live
                npairs = ni // 2
                for h in range(QH):
                    acc_av = avp.tile([128, 512], f32, tag="av")
                    acc_dn = dnp.tile([1, 512], f32, tag="dn")
                    for pr in range(npairs):
                        s2 = sp.tile([128, 1024], f32, tag="s")
                        for b in range(2):
                            i = 2 * pr + b
                            # scaled K for full-region rows (j>=2), keys<640
                            if j >= 2 and i < 5:
                                kt = krot_s[:, i * 128:(i + 1) * 128]
                            else:
                                kt = krot[:, i * 128:(i + 1) * 128]
                            nc.tensor.matmul(s2[:, b * 512:(b + 1) * 512],
                                             kt, qrot[h][:, sq],
                                             start=True, stop=True)
                        if j == 1:
                            # partial region rows: queries 611.. are cols 99..
                            for b in range(2):
                                i = 2 * pr + b
                                if i < 5:
                                    c0 = b * 512 + 99
                                    nc.vector.tensor_scalar_mul(
                                        s2[:, c0:(b + 1) * 512],
                                        s2[:, c0:(b + 1) * 512],
                                        ksT[:, i:i + 1])
                        e2 = att_sb.tile([128, 1024], bf, tag="e")
                        nc.scalar.activation(e2[:], s2[:], EXP)
                        for b in range(2):
                            i = 2 * pr + b
                            delta = i - 4 * j
                            if delta >= 0:
                                nc.vector.tensor_mul(
                                    e2[:, b * 512:(b + 1) * 512],
                                    e2[:, b * 512:(b + 1) * 512],
                                    masks[:, delta * 512:(delta + 1) * 512])
                        for b in range(2):
                            i = 2 * pr + b
                            st = (i == 0)
                            sp_l = (i == ni - 1)
                            eb = e2[:, b * 512:(b + 1) * 512]
                            nc.tensor.matmul(acc_dn[:], ones_col[:], eb,
                                             start=st, stop=sp_l)
                            nc.tensor.matmul(acc_av[:],
                                             vnat[:, i * 128:(i + 1) * 128],
                                             eb, start=st, stop=sp_l)
                    # 1/denom on DVE, broadcast via K=1 matmul
                    rec2 = nrm.tile([1, 512], f32r, tag="rec2")
                    with nc.allow_low_precision(
                            reason="f32r is bitwise float32; full precision"):
                        nc.vector.reciprocal(rec2[:], acc_dn[:])
                    rb_ps = rbp.tile([128, 512], f32, tag="rb")
                    nc.tensor.matmul(rb_ps[:], ones_row[:], rec2[:],
                                     start=True, stop=True)
                    rb_sb = nrm.tile([128, 512], f32, tag="rb_sb")
                    nc.vector.tensor_copy(rb_sb[:], rb_ps[:])
                    nc.vector.tensor_mul(attn[h][:, sq], acc_av[:], rb_sb[:])

        # ---------------- Phase C: o_proj (out.T layout) -------------------
        with tc.tile_pool(name="op", bufs=4, space="PSUM") as op, \
             tc.tile_pool(name="ost", bufs=2) as ost:
            for j in range(J):
                sq = slice(j * 512, (j + 1) * 512)
                for cq in range(DCH // 4):      # c in groups of 4
                    o_sb = ost.tile([128, 4 * 512], bf, tag="o_sb")
                    for cb in range(4):
                        c = cq * 4 + cb
                        o_ps = op.tile([128, 512], f32, tag="o")
                        for h in range(QH):
                            nc.tensor.matmul(
                                o_ps[:],
                                wo_sb[:, h * D + c * 128:h * D + (c + 1) * 128],
                                attn[h][:, sq], start=(h == 0), stop=(h == QH - 1))
                        nc.vector.tensor_copy(
                            o_sb[:, cb * 512:(cb + 1) * 512], o_ps[:])
                    nc.sync.dma_start(out[j, cq], o_sb[:])

    # Split multi-wait instructions onto standalone EventSemaphore insts.
    import bass_rust
    bass_rust.generate_event_semaphores(nc)
    return nc


def _get_compiled():
    if "nc" not in _CACHE:
        _CACHE["nc"] = _build_bass()
        _CACHE["const"] = _host_constants()
    return _CACHE["nc"], _CACHE["const"]


def kernel(hidden_states, wq, wk, wv, wo, _trace=False):
    from concourse.bass_utils import run_bass_kernel_spmd

    nc, cst = _get_compiled()
    bf16 = _bf16()

    x = np.asarray(hidden_states, dtype=np.float32).reshape(S, D)
    xT = x.T                                             # [D, S]
    # xp[j, g, p, dd*512+f] = xT[128*(8g+dd)+p, 512j+f]
    t = xT.reshape(G, GD, 128, J, 512)                   # [g, dd, p, j, f]
    xpk = np.ascontiguousarray(
        t.transpose(3, 0, 2, 1, 4).reshape(J, G, 128, GD * 512)).astype(bf16)
    wq = np.asarray(wq, dtype=np.float32)
    wk = np.asarray(wk, dtype=np.float32)
    wv = np.asarray(wv, dtype=np.float32)
    wo = np.asarray(wo, dtype=np.float32)
    scale = 1.0 / math.sqrt(HD)

    in_maps = []
    for d in range(NCORES):
        wq_d = wq[d * QH * HD:(d + 1) * QH * HD] * scale      # [512, D]
        wqT = wq_d.T                                          # [4096, 512]
        wqpk = np.ascontiguousarray(
            wqT.reshape(G, GD, 128, 512).transpose(0, 2, 1, 3)
            .reshape(G, 128, GD * 512)).astype(bf16)
        wkT = wk[d * HD:(d + 1) * HD].T.reshape(G, GD, 128, 128)
        wvT = wv[d * HD:(d + 1) * HD].T.reshape(G, GD, 128, 128)
        kv = np.concatenate([wkT, wvT], axis=-1)              # [G, GD, 128, 256]
        wkvpk = np.ascontiguousarray(
            kv.transpose(0, 2, 1, 3).reshape(G, 128, GD * 256)).astype(bf16)
        woT = wo[:, d * QH * HD:(d + 1) * QH * HD].T          # [512, 4096]
        wopk = np.ascontiguousarray(
            woT.reshape(QH, 128, D).transpose(1, 0, 2)
            .reshape(128, QH * D)).astype(bf16)
        in_maps.append({
            "xp": xpk,
            "wqp": wqpk,
            "wkvp": wkvpk,
            "wop": wopk,
            "cosT": cst["cosT"], "sinT": cst["sinT"],
            "rmat": cst["rmat"], "ident": cst["ident"],
            "masks": cst["masks"], "ksT": cst["ksT"], "ksB": cst["ksB"],
            "ones_col": cst["ones_col"],
            "ones_row": cst["ones_row"],
        })

    res = run_bass_kernel_spmd(nc, in_maps, core_ids=list(range(NCORES)),
                               trace=_trace)
    acc = res.results[0]["out"].astype(np.float32)
    for d in range(1, NCORES):
        acc += res.results[d]["out"].astype(np.float32)
    # out_t[j, cq, p, cb*512+f] = out.T[128*(4cq+cb)+p, 512j+f]
    acc = acc.reshape(J, DCH // 4, 128, 4, 512)          # [j, cq, p, cb, f]
    outp = acc.transpose(0, 4, 1, 3, 2).reshape(S, D)    # [512j+f, 128(4cq+cb)+p]
    outp = outp.reshape(1, S, D).astype(np.float32)
    if _trace:
        _CACHE["last_results"] = res
    return outp


# revision 13
# speedup vs baseline: 1.3749x; 1.0914x over previous
"""Trainium2 Bass kernel for nn_AttnAdapter: GQA attention with RoPE,
region-based enhance/suppress score scaling, causal mask, o_proj.

Sharding: tensor-parallel over heads across 8 NeuronCores. Core d holds
q-heads 4d..4d+3 (wq rows), kv-head d (wk/wv rows), and wo columns
512d..512(d+1). Each core computes a full [S, D] partial of the output;
the host sums the 8 partials (the TP all-reduce, done at unshard time).

v2: all matmuls in bf16 (fp32 PSUM accumulate), weights resident in
SBUF (single load), exp batched over [128,1024] PSUM pairs, region
scale folded into pre-scaled K columns, reciprocal on DVE, all PSUM
evictions on DVE, batched DMAs.
"""

import math

import numpy as np

# ---- problem constants (hardcoded; kernel.py must be self-contained) ----
S = 2048          # sequence length
D = 4096          # model dim
HD = 128          # head dim
NCORES = 8
QH = 4            # q heads per core
SYS_LEN, IMG_LEN = 35, 576
BOUND = SYS_LEN + IMG_LEN          # 611
ENH, SUP = 1.5, 0.5
ROPE_BASE = 10000.0

J = 4             # sq tiles of 512
NSK = 16          # sk tiles of 128
DCH = 32          # D chunks of 128
G = 4             # d-groups of 8 chunks
GD = 8            # d-chunks per group

_CACHE = {}


def _bf16():
    import ml_dtypes
    return ml_dtypes.bfloat16


def _host_constants():
    bf16 = _bf16()
    inv_freq = 1.0 / (ROPE_BASE ** (np.arange(0, HD, 2, dtype=np.float32) / HD))
    pos = np.arange(S, dtype=np.float32)
    freqs = pos[:, None] * inv_freq[None, :]              # [S, 64]
    emb = np.concatenate([freqs, freqs], axis=-1)         # [S, 128]
    cosT = np.ascontiguousarray(np.cos(emb).T.astype(np.float32))  # [128, S]
    sinT = np.ascontiguousarray(np.sin(emb).T.astype(np.float32))

    # rotate_half as a matmul: rot = R @ q (in [hd, s] layout).
    # matmul(out, lhsT, rhs) = lhsT.T @ rhs, so feed RT = R.T.
    RT = np.zeros((HD, HD), dtype=np.float32)
    half = HD // 2
    for c in range(half):
        RT[c + half, c] = -1.0      # rot[c] = -q[c+64]
    for c in range(half, HD):
        RT[c - half, c] = 1.0       # rot[c] = q[c-64]

    ident = np.eye(HD, dtype=np.float32)

    # Diagonal-tile causal masks, T layout [sk 128, sq 512]:
    # tile (i=4j+delta, j): valid (keep) iff sq >= sk  <=>  f >= 128*delta + p
    masks = np.zeros((HD, 4 * 512), dtype=np.float32)
    p = np.arange(128)[:, None]
    f = np.arange(512)[None, :]
    for delta in range(4):
        masks[:, delta * 512:(delta + 1) * 512] = (f >= 128 * delta + p)

    kpos = np.arange(S)
    key_scale = np.where(kpos < SYS_LEN, SUP,
                         np.where(kpos < BOUND, ENH, 1.0)).astype(np.float32)
    # per-partition scale per sk-tile (j=1 partial-row DVE path)
    ksT = np.ascontiguousarray(key_scale[:5 * 128].reshape(5, 128).T)  # [128, 5]
    # per-key-column broadcast (for pre-scaling krot columns, j>=2 path)
    ksB = np.ascontiguousarray(
        np.broadcast_to(key_scale[:640][None, :], (HD, 640)))          # [128, 640]

    return dict(
        cosT=cosT, sinT=sinT,
        rmat=RT.astype(bf16), ident=ident.astype(bf16),
        masks=masks.astype(bf16), ksT=ksT, ksB=ksB.astype(bf16),
        ones_col=np.ones((HD, 1), dtype=bf16),
    )


def _build_bass():
    import concourse.bass as bass
    import concourse.mybir as mybir
    from concourse.tile import TileContext
    from contextlib import ExitStack

    f32 = mybir.dt.float32
    f32r = mybir.dt.float32r
    bf = mybir.dt.bfloat16

    nc = bass.Bass()
    # x packed per (j, g): [128, GD*512] with [p, dd*512+f] = x.T[128(8g+dd)+p, 512j+f]
    xp = nc.dram_tensor("xp", [J, G, 128, GD * 512], bf, kind="ExternalInput")
    # wq packed per g: [128, GD*512] with [p, dd*512+f] = wq_d.T[128(8g+dd)+p, f]
    wqp = nc.dram_tensor("wqp", [G, 128, GD * 512], bf, kind="ExternalInput")
    # wk/wv packed per g: [128, GD*256], cols dd*256+0:128 = wkT, 128:256 = wvT
    wkvp = nc.dram_tensor("wkvp", [G, 128, GD * 256], bf, kind="ExternalInput")
    # wo packed: [128, 4*4096] with [p, h*4096+f] = wo[:, core].T[128h+p, f]
    wop = nc.dram_tensor("wop", [128, QH * D], bf, kind="ExternalInput")
    cosT_d = nc.dram_tensor("cosT", [HD, S], f32, kind="ExternalInput")
    sinT_d = nc.dram_tensor("sinT", [HD, S], f32, kind="ExternalInput")
    rmat_d = nc.dram_tensor("rmat", [HD, HD], bf, kind="ExternalInput")
    ident_d = nc.dram_tensor("ident", [HD, HD], bf, kind="ExternalInput")
    masks_d = nc.dram_tensor("masks", [HD, 4 * 512], bf, kind="ExternalInput")
    ksT_d = nc.dram_tensor("ksT", [HD, 5], f32, kind="ExternalInput")
    ksB_d = nc.dram_tensor("ksB", [HD, 640], bf, kind="ExternalInput")
    onesc_d = nc.dram_tensor("ones_col", [HD, 1], bf, kind="ExternalInput")
    # per-(j,h) softmax 1/denom staging for the DMA partition-broadcast
    rscr = nc.dram_tensor("rscr", [J * QH, 512], f32, kind="Internal")
    # out.T tiles: out_t[j, cq, p, cb*512+f] = out.T[128*(4cq+cb)+p, 512j+f]
    out = nc.dram_tensor("out", [J, DCH // 4, 128, 4 * 512], bf,
                         kind="ExternalOutput")

    EXP = mybir.ActivationFunctionType.Exp

    with TileContext(nc) as tc, ExitStack() as ctx:
        # ---- resident weights (single load, used all phases) ----
        # Order matters for startup latency: the first matmuls need only
        # wq/wkv chunk g=0 and x(j=0,g=0); everything else trails behind.
        wsb = ctx.enter_context(tc.tile_pool(name="wsb", bufs=1))
        wq_sb = wsb.tile([128, DCH * 512], bf)
        wkv_sb = wsb.tile([128, DCH * 256], bf)
        nc.sync.dma_start(wq_sb[:, 0:GD * 512], wqp[0])
        nc.sync.dma_start(wkv_sb[:, 0:GD * 256], wkvp[0])
        xgp = ctx.enter_context(tc.tile_pool(name="xg", bufs=2))
        xt0 = xgp.tile([128, GD * 512], bf, tag="xt")
        nc.sync.dma_start(xt0[:], xp[0, 0])
        for g in range(1, G):
            nc.sync.dma_start(wq_sb[:, g * GD * 512:(g + 1) * GD * 512], wqp[g])
            nc.sync.dma_start(wkv_sb[:, g * GD * 256:(g + 1) * GD * 256], wkvp[g])

        const = ctx.enter_context(tc.tile_pool(name="const", bufs=1))
        cosT = const.tile([HD, S], f32)
        nc.sync.dma_start(cosT[:], cosT_d[:, :])
        sinT = const.tile([HD, S], f32)
        nc.sync.dma_start(sinT[:], sinT_d[:, :])
        rmat = const.tile([HD, HD], bf)
        nc.sync.dma_start(rmat[:], rmat_d[:, :])
        ident = const.tile([HD, HD], bf)
        nc.sync.dma_start(ident[:], ident_d[:, :])
        masks = const.tile([HD, 4 * 512], bf)
        nc.sync.dma_start(masks[:], masks_d[:, :])
        ksT = const.tile([HD, 5], f32)
        nc.sync.dma_start(ksT[:], ksT_d[:, :])
        ksB = const.tile([HD, 640], bf)
        nc.sync.dma_start(ksB[:], ksB_d[:, :])
        ones_col = const.tile([HD, 1], bf)
        nc.sync.dma_start(ones_col[:], onesc_d[:, :])
        wo_sb = const.tile([128, QH * D], bf)
        nc.sync.dma_start(wo_sb[:], wop[:, :])

        persist = ctx.enter_context(tc.tile_pool(name="persist", bufs=1))
        qrot = [persist.tile([HD, S], bf, name=f"qrot{m}") for m in range(QH)]
        krot = persist.tile([HD, S], bf)
        krot_s = persist.tile([HD, 640], bf)     # region-scaled K cols (keys<640)
        vnat = persist.tile([HD, NSK * HD], bf)  # tile i at cols i*128
        attn = [persist.tile([HD, S], bf, name=f"attn{h}") for h in range(QH)]

        # ---------------- Phase A: projections + RoPE + V transpose --------
        with tc.tile_pool(name="accp", bufs=1, space="PSUM") as accp, \
             tc.tile_pool(name="ropep", bufs=1, space="PSUM") as ropep, \
             tc.tile_pool(name="vtp", bufs=1, space="PSUM") as vtp, \
             tc.tile_pool(name="stage", bufs=2) as stage:
            for j in range(J):
                sq = slice(j * 512, (j + 1) * 512)
                accs = [accp.tile([128, 512], f32, name=f"acc{m}") for m in range(6)]
                for g in range(G):
                    if j == 0 and g == 0:
                        xt = xt0
                    else:
                        xt = xgp.tile([128, GD * 512], bf, tag="xt")
                        nc.sync.dma_start(xt[:], xp[j, g])
                    for dd in range(GD):
                        d = g * GD + dd
                        xs = xt[:, dd * 512:(dd + 1) * 512]
                        st = (d == 0)
                        sp_ = (d == DCH - 1)
                        qb = d * 512
                        for m in range(QH):
                            nc.tensor.matmul(
                                accs[m][:], wq_sb[:, qb + m * 128:qb + (m + 1) * 128],
                                xs, start=st, stop=sp_)
                        kb = d * 256
                        nc.tensor.matmul(accs[4][:], wkv_sb[:, kb:kb + 128],
                                         xs, start=st, stop=sp_)
                        nc.tensor.matmul(accs[5][:], wkv_sb[:, kb + 128:kb + 256],
                                         xs, start=st, stop=sp_)

                # RoPE for q tiles and k tile
                for m in range(5):
                    dst = qrot[m][:, sq] if m < QH else krot[:, sq]
                    q_sb = stage.tile([128, 512], bf, tag="q_sb")
                    nc.scalar.copy(q_sb[:], accs[m][:])
                    rot_ps = ropep.tile([128, 512], f32, tag="rope_ps")
                    nc.tensor.matmul(rot_ps[:], rmat[:], q_sb[:],
                                     start=True, stop=True)
                    t1 = stage.tile([128, 512], f32, tag="t1")
                    nc.vector.tensor_mul(t1[:], accs[m][:], cosT[:, sq])
                    t2 = stage.tile([128, 512], f32, tag="t2")
                    nc.vector.tensor_mul(t2[:], rot_ps[:], sinT[:, sq])
                    nc.vector.tensor_add(dst, t1[:], t2[:])
                # region-scaled K columns (keys < 640 only)
                if j == 0:
                    nc.vector.tensor_mul(krot_s[:, 0:512], krot[:, 0:512],
                                         ksB[:, 0:512])
                elif j == 1:
                    nc.vector.tensor_mul(krot_s[:, 512:640], krot[:, 512:640],
                                         ksB[:, 512:640])

                # V: copy to SBUF, transpose 128x128 blocks into vnat
                v_sb = stage.tile([128, 512], bf, tag="v_sb")
                nc.scalar.copy(v_sb[:], accs[5][:])
                vt_ps = vtp.tile([128, 512], bf, tag="vt_ps")
                for b in range(4):
                    nc.tensor.transpose(vt_ps[:, b * 128:(b + 1) * 128],
                                        v_sb[:, b * 128:(b + 1) * 128], ident[:])
                nc.vector.tensor_copy(
                    vnat[:, (4 * j) * 128:(4 * j + 4) * 128], vt_ps[:])

        # ---------------- Phase B: attention ------------------------------
        # Software-pipelined emission: the PE queue is in-order, so the
        # scores matmuls of pair t+1 must be EMITTED before the dn/av
        # matmuls of pair t (which wait on exp(t) on ACT) or PE idles for
        # every exp. Finalize (normalize) of head hg is emitted one pair
        # into head hg+1 so its DVE chain never stalls PE.
        with tc.tile_pool(name="att_sb", bufs=3) as att_sb, \
             tc.tile_pool(name="sp", bufs=2, space="PSUM") as sp, \
             tc.tile_pool(name="avp", bufs=2, space="PSUM") as avp, \
             tc.tile_pool(name="dnp", bufs=2, space="PSUM") as dnp, \
             tc.tile_pool(name="nrm", bufs=2) as nrm:
            items = []
            for j in range(J):
                for h in range(QH):
                    npairs = (4 * j + 4) // 2
                    for pr in range(npairs):
                        items.append((j, h, pr, npairs))

            def emit_scores(it):
                j, h, pr, npairs = it
                sq = slice(j * 512, (j + 1) * 512)
                s2 = sp.tile([128, 1024], f32, tag="s")
                for b in range(2):
                    i = 2 * pr + b
                    # scaled K for full-region rows (j>=2), keys<640
                    if j >= 2 and i < 5:
                        kt = krot_s[:, i * 128:(i + 1) * 128]
                    else:
                        kt = krot[:, i * 128:(i + 1) * 128]
                    nc.tensor.matmul(s2[:, b * 512:(b + 1) * 512],
                                     kt, qrot[h][:, sq], start=True, stop=True)
                if j == 1:
                    # partial region rows: queries 611.. are cols 99..
                    for b in range(2):
                        i = 2 * pr + b
                        if i < 5:
                            c0 = b * 512 + 99
                            nc.vector.tensor_scalar_mul(
                                s2[:, c0:(b + 1) * 512],
                                s2[:, c0:(b + 1) * 512], ksT[:, i:i + 1])
                return s2

            state = {}

            def emit_tail(it, s2):
                j, h, pr, npairs = it
                ni = 4 * j + 4
                if pr == 0:
                    state[(j, h)] = (
                        avp.tile([128, 512], f32, tag="av", name="acc_av"),
                        dnp.tile([1, 512], f32, tag="dn", name="acc_dn"))
                acc_av, acc_dn = state[(j, h)]
                e2 = att_sb.tile([128, 1024], bf, tag="e")
                nc.scalar.activation(e2[:], s2[:], EXP)
                for b in range(2):
                    i = 2 * pr + b
                    delta = i - 4 * j
                    if delta >= 0:
                        nc.vector.tensor_mul(
                            e2[:, b * 512:(b + 1) * 512],
                            e2[:, b * 512:(b + 1) * 512],
                            masks[:, delta * 512:(delta + 1) * 512])
                for b in range(2):
                    i = 2 * pr + b
                    st = (i == 0)
                    sp_l = (i == ni - 1)
                    eb = e2[:, b * 512:(b + 1) * 512]
                    nc.tensor.matmul(acc_dn[:], ones_col[:], eb,
                                     start=st, stop=sp_l)
                    nc.tensor.matmul(acc_av[:],
                                     vnat[:, i * 128:(i + 1) * 128],
                                     eb, start=st, stop=sp_l)

            def emit_finalize(j, h):
                sq = slice(j * 512, (j + 1) * 512)
                acc_av, acc_dn = state.pop((j, h))
                # 1/x = exp(-ln(x)) on ACT; partition-broadcast [1,512] ->
                # [128,512] via a DRAM round-trip (0-stride DRAM source AP)
                # so no PE matmul / PSUM bank is needed.
                lrec = nrm.tile([1, 512], f32, tag="lrec")
                nc.scalar.activation(lrec[:], acc_dn[:],
                                     mybir.ActivationFunctionType.Ln)
                rec2 = nrm.tile([1, 512], f32, tag="rec2")
                nc.scalar.activation(rec2[:], lrec[:], EXP, scale=-1.0)
                hg = j * QH + h
                nc.sync.dma_start(rscr[hg:hg + 1, :], rec2[:])
                rb_sb = nrm.tile([128, 512], f32, tag="rb_sb")
                nc.sync.dma_start(rb_sb[:],
                                  rscr[hg:hg + 1, :].broadcast_to((128, 512)))
                nc.vector.tensor_mul(attn[h][:, sq], acc_av[:], rb_sb[:])

            s2_cur = emit_scores(items[0])
            pending = None
            for t, it in enumerate(items):
                s2_next = emit_scores(items[t + 1]) if t + 1 < len(items) else None
                emit_tail(it, s2_cur)
                s2_cur = s2_next
                if pending is not None and pending != (it[0], it[1]):
                    emit_finalize(*pending)
                    pending = None
                if it[2] == it[3] - 1:       # last pair of this (j, h)
                    pending = (it[0], it[1])
            emit_finalize(*pending)

        # ---------------- Phase C: o_proj (out.T layout) -------------------
        with tc.tile_pool(name="op", bufs=4, space="PSUM") as op, \
             tc.tile_pool(name="ost", bufs=2) as ost:
            for j in range(J):
                sq = slice(j * 512, (j + 1) * 512)
                for cq in range(DCH // 4):      # c in groups of 4
                    o_sb = ost.tile([128, 4 * 512], bf, tag="o_sb")
                    for cb in range(4):
                        c = cq * 4 + cb
                        o_ps = op.tile([128, 512], f32, tag="o")
                        for h in range(QH):
                            nc.tensor.matmul(
                                o_ps[:],
                                wo_sb[:, h * D + c * 128:h * D + (c + 1) * 128],
                                attn[h][:, sq], start=(h == 0), stop=(h == QH - 1))
                        if cb % 2 == 0:
                            nc.scalar.copy(o_sb[:, cb * 512:(cb + 1) * 512],
                                           o_ps[:])
                        else:
                            nc.vector.tensor_copy(
                                o_sb[:, cb * 512:(cb + 1) * 512], o_ps[:])
                    nc.sync.dma_start(out[j, cq], o_sb[:])

    # Split multi-wait instructions onto standalone EventSemaphore insts.
    import bass_rust
    bass_rust.generate_event_semaphores(nc)
    return nc


def _get_compiled():
    if "nc" not in _CACHE:
        _CACHE["nc"] = _build_bass()
        _CACHE["const"] = _host_constants()
    return _CACHE["nc"], _CACHE["const"]


def kernel(hidden_states, wq, wk, wv, wo, _trace=False):
    from concourse.bass_utils import run_bass_kernel_spmd

    nc, cst = _get_compiled()
    bf16 = _bf16()

    x = np.asarray(hidden_states, dtype=np.float32).reshape(S, D)
    xT = x.T                                             # [D, S]
    # xp[j, g, p, dd*512+f] = xT[128*(8g+dd)+p, 512j+f]
    t = xT.reshape(G, GD, 128, J, 512)                   # [g, dd, p, j, f]
    xpk = np.ascontiguousarray(
        t.transpose(3, 0, 2, 1, 4).reshape(J, G, 128, GD * 512)).astype(bf16)
    wq = np.asarray(wq, dtype=np.float32)
    wk = np.asarray(wk, dtype=np.float32)
    wv = np.asarray(wv, dtype=np.float32)
    wo = np.asarray(wo, dtype=np.float32)
    scale = 1.0 / math.sqrt(HD)

    in_maps = []
    for d in range(NCORES):
        wq_d = wq[d * QH * HD:(d + 1) * QH * HD] * scale      # [512, D]
        wqT = wq_d.T                                          # [4096, 512]
        wqpk = np.ascontiguousarray(
            wqT.reshape(G, GD, 128, 512).transpose(0, 2, 1, 3)
            .reshape(G, 128, GD * 512)).astype(bf16)
        wkT = wk[d * HD:(d + 1) * HD].T.reshape(G, GD, 128, 128)
        wvT = wv[d * HD:(d + 1) * HD].T.reshape(G, GD, 128, 128)
        kv = np.concatenate([wkT, wvT], axis=-1)              # [G, GD, 128, 256]
        wkvpk = np.ascontiguousarray(
            kv.transpose(0, 2, 1, 3).reshape(G, 128, GD * 256)).astype(bf16)
        woT = wo[:, d * QH * HD:(d + 1) * QH * HD].T          # [512, 4096]
        wopk = np.ascontiguousarray(
            woT.reshape(QH, 128, D).transpose(1, 0, 2)
            .reshape(128, QH * D)).astype(bf16)
        in_maps.append({
            "xp": xpk,
            "wqp": wqpk,
            "wkvp": wkvpk,
            "wop": wopk,
            "cosT": cst["cosT"], "sinT": cst["sinT"],
            "rmat": cst["rmat"], "ident": cst["ident"],
            "masks": cst["masks"], "ksT": cst["ksT"], "ksB": cst["ksB"],
            "ones_col": cst["ones_col"],
        })

    res = run_bass_kernel_spmd(nc, in_maps, core_ids=list(range(NCORES)),
                               trace=_trace)
    acc = res.results[0]["out"].astype(np.float32)
    for d in range(1, NCORES):
        acc += res.results[d]["out"].astype(np.float32)
    # out_t[j, cq, p, cb*512+f] = out.T[128*(4cq+cb)+p, 512j+f]
    acc = acc.reshape(J, DCH // 4, 128, 4, 512)          # [j, cq, p, cb, f]
    outp = acc.transpose(0, 4, 1, 3, 2).reshape(S, D)    # [512j+f, 128(4cq+cb)+p]
    outp = outp.reshape(1, S, D).astype(np.float32)
    if _trace:
        _CACHE["last_results"] = res
    return outp


# revision 18
# speedup vs baseline: 1.4508x; 1.0553x over previous
"""Trainium2 Bass kernel for nn_AttnAdapter: GQA attention with RoPE,
region-based enhance/suppress score scaling, causal mask, o_proj.

Sharding: tensor-parallel over heads across 8 NeuronCores. Core d holds
q-heads 4d..4d+3 (wq rows), kv-head d (wk/wv rows), and wo columns
512d..512(d+1). Each core computes a full [S, D] partial of the output;
the host sums the 8 partials (the TP all-reduce, done at unshard time).

v2: all matmuls in bf16 (fp32 PSUM accumulate), weights resident in
SBUF (single load), exp batched over [128,1024] PSUM pairs, region
scale folded into pre-scaled K columns, reciprocal on DVE, all PSUM
evictions on DVE, batched DMAs.
"""

import math

import numpy as np

# ---- problem constants (hardcoded; kernel.py must be self-contained) ----
S = 2048          # sequence length
D = 4096          # model dim
HD = 128          # head dim
NCORES = 8
QH = 4            # q heads per core
SYS_LEN, IMG_LEN = 35, 576
BOUND = SYS_LEN + IMG_LEN          # 611
ENH, SUP = 1.5, 0.5
ROPE_BASE = 10000.0

J = 4             # sq tiles of 512
NSK = 16          # sk tiles of 128
DCH = 32          # D chunks of 128
G = 4             # d-groups of 8 chunks
GD = 8            # d-chunks per group

_CACHE = {}


def _bf16():
    import ml_dtypes
    return ml_dtypes.bfloat16


def _host_constants():
    bf16 = _bf16()
    inv_freq = 1.0 / (ROPE_BASE ** (np.arange(0, HD, 2, dtype=np.float32) / HD))
    pos = np.arange(S, dtype=np.float32)
    freqs = pos[:, None] * inv_freq[None, :]              # [S, 64]
    emb = np.concatenate([freqs, freqs], axis=-1)         # [S, 128]
    cosT = np.ascontiguousarray(np.cos(emb).T.astype(np.float32))  # [128, S]
    sinT = np.ascontiguousarray(np.sin(emb).T.astype(np.float32))

    # rotate_half as a matmul: rot = R @ q (in [hd, s] layout).
    # matmul(out, lhsT, rhs) = lhsT.T @ rhs, so feed RT = R.T.
    RT = np.zeros((HD, HD), dtype=np.float32)
    half = HD // 2
    for c in range(half):
        RT[c + half, c] = -1.0      # rot[c] = -q[c+64]
    for c in range(half, HD):
        RT[c - half, c] = 1.0       # rot[c] = q[c-64]

    ident = np.eye(HD, dtype=np.float32)

    # Diagonal-tile causal masks, T layout [sk 128, sq 512]:
    # tile (i=4j+delta, j): valid (keep) iff sq >= sk  <=>  f >= 128*delta + p
    masks = np.zeros((HD, 4 * 512), dtype=np.float32)
    p = np.arange(128)[:, None]
    f = np.arange(512)[None, :]
    for delta in range(4):
        masks[:, delta * 512:(delta + 1) * 512] = (f >= 128 * delta + p)

    kpos = np.arange(S)
    key_scale = np.where(kpos < SYS_LEN, SUP,
                         np.where(kpos < BOUND, ENH, 1.0)).astype(np.float32)
    # per-partition scale per sk-tile (j=1 partial-row DVE path)
    ksT = np.ascontiguousarray(key_scale[:5 * 128].reshape(5, 128).T)  # [128, 5]
    # per-key-column broadcast (for pre-scaling krot columns, j>=2 path)
    ksB = np.ascontiguousarray(
        np.broadcast_to(key_scale[:640][None, :], (HD, 640)))          # [128, 640]

    return dict(
        cosT=cosT, sinT=sinT,
        rmat=RT.astype(bf16), ident=ident.astype(bf16),
        masks=masks.astype(bf16), ksT=ksT, ksB=ksB.astype(bf16),
        ones_col=np.ones((HD, 1), dtype=bf16),
    )


def _build_bass():
    import concourse.bass as bass
    import concourse.mybir as mybir
    from concourse.tile import TileContext
    from contextlib import ExitStack

    f32 = mybir.dt.float32
    f32r = mybir.dt.float32r
    bf = mybir.dt.bfloat16

    nc = bass.Bass()
    # x packed per (j, g): [128, GD*512] with [p, dd*512+f] = x.T[128(8g+dd)+p, 512j+f]
    xp = nc.dram_tensor("xp", [J, G, 128, GD * 512], bf, kind="ExternalInput")
    # wq packed per g: [128, GD*512] with [p, dd*512+f] = wq_d.T[128(8g+dd)+p, f]
    wqp = nc.dram_tensor("wqp", [G, 128, GD * 512], bf, kind="ExternalInput")
    # wk/wv packed per g: [128, GD*256], cols dd*256+0:128 = wkT, 128:256 = wvT
    wkvp = nc.dram_tensor("wkvp", [G, 128, GD * 256], bf, kind="ExternalInput")
    # wo packed: [128, 4*4096] with [p, h*4096+f] = wo[:, core].T[128h+p, f]
    wop = nc.dram_tensor("wop", [128, QH * D], bf, kind="ExternalInput")
    cosT_d = nc.dram_tensor("cosT", [HD, S], f32, kind="ExternalInput")
    sinT_d = nc.dram_tensor("sinT", [HD, S], f32, kind="ExternalInput")
    rmat_d = nc.dram_tensor("rmat", [HD, HD], bf, kind="ExternalInput")
    ident_d = nc.dram_tensor("ident", [HD, HD], bf, kind="ExternalInput")
    masks_d = nc.dram_tensor("masks", [HD, 4 * 512], bf, kind="ExternalInput")
    ksT_d = nc.dram_tensor("ksT", [HD, 5], f32, kind="ExternalInput")
    ksB_d = nc.dram_tensor("ksB", [HD, 640], bf, kind="ExternalInput")
    onesc_d = nc.dram_tensor("ones_col", [HD, 1], bf, kind="ExternalInput")
    # per-(j,h) softmax 1/denom staging for the DMA partition-broadcast
    rscr = nc.dram_tensor("rscr", [J * QH, 512], f32, kind="Internal")
    # out.T tiles: out_t[j, cq, p, cb*512+f] = out.T[128*(4cq+cb)+p, 512j+f]
    out = nc.dram_tensor("out", [J, DCH // 4, 128, 4 * 512], bf,
                         kind="ExternalOutput")

    EXP = mybir.ActivationFunctionType.Exp

    with TileContext(nc) as tc, ExitStack() as ctx:
        # ---- resident weights (single load, used all phases) ----
        # Order matters for startup latency: the first matmuls need only
        # wq/wkv chunk g=0 and x(j=0,g=0); everything else trails behind.
        wsb = ctx.enter_context(tc.tile_pool(name="wsb", bufs=1))
        wq_sb = wsb.tile([128, DCH * 512], bf)
        wkv_sb = wsb.tile([128, DCH * 256], bf)
        nc.sync.dma_start(wq_sb[:, 0:GD * 512], wqp[0])
        nc.sync.dma_start(wkv_sb[:, 0:GD * 256], wkvp[0])
        xgp = ctx.enter_context(tc.tile_pool(name="xg", bufs=3))
        xpre = {}
        xt0 = xgp.tile([128, GD * 512], bf, tag="xt", name="xt0")
        nc.sync.dma_start(xt0[:], xp[0, 0])
        xpre[(0, 0)] = xt0
        for g in range(1, G):
            nc.sync.dma_start(wq_sb[:, g * GD * 512:(g + 1) * GD * 512], wqp[g])
            nc.sync.dma_start(wkv_sb[:, g * GD * 256:(g + 1) * GD * 256], wkvp[g])
            if g <= 2:
                xt_n = xgp.tile([128, GD * 512], bf, tag="xt", name=f"xt{g}")
                nc.sync.dma_start(xt_n[:], xp[0, g])
                xpre[(0, g)] = xt_n

        const = ctx.enter_context(tc.tile_pool(name="const", bufs=1))
        cosT = const.tile([HD, S], f32)
        nc.sync.dma_start(cosT[:], cosT_d[:, :])
        sinT = const.tile([HD, S], f32)
        nc.sync.dma_start(sinT[:], sinT_d[:, :])
        rmat = const.tile([HD, HD], bf)
        nc.sync.dma_start(rmat[:], rmat_d[:, :])
        ident = const.tile([HD, HD], bf)
        nc.sync.dma_start(ident[:], ident_d[:, :])
        masks = const.tile([HD, 4 * 512], bf)
        nc.sync.dma_start(masks[:], masks_d[:, :])
        ksT = const.tile([HD, 5], f32)
        nc.sync.dma_start(ksT[:], ksT_d[:, :])
        ksB = const.tile([HD, 640], bf)
        nc.sync.dma_start(ksB[:], ksB_d[:, :])
        ones_col = const.tile([HD, 1], bf)
        nc.sync.dma_start(ones_col[:], onesc_d[:, :])
        wo_sb = const.tile([128, QH * D], bf)
        nc.sync.dma_start(wo_sb[:], wop[:, :])

        persist = ctx.enter_context(tc.tile_pool(name="persist", bufs=1))
        qrot = [persist.tile([HD, S], bf, name=f"qrot{m}") for m in range(QH)]
        krot = persist.tile([HD, S], bf)
        krot_s = persist.tile([HD, 640], bf)     # region-scaled K cols (keys<640)
        vnat = persist.tile([HD, NSK * HD], bf)  # tile i at cols i*128
        attn = [persist.tile([HD, S], bf, name=f"attn{h}") for h in range(QH)]

        # ---------------- Phase A: projections + RoPE + V transpose --------
        with tc.tile_pool(name="accp", bufs=1, space="PSUM") as accp, \
             tc.tile_pool(name="ropep", bufs=1, space="PSUM") as ropep, \
             tc.tile_pool(name="vtp", bufs=1, space="PSUM") as vtp, \
             tc.tile_pool(name="stage", bufs=2) as stage:
            for j in range(J):
                sq = slice(j * 512, (j + 1) * 512)
                accs = [accp.tile([128, 512], f32, name=f"acc{m}") for m in range(6)]
                for g in range(G):
                    if (j, g) in xpre:
                        xt = xpre.pop((j, g))
                    else:
                        xt = xgp.tile([128, GD * 512], bf, tag="xt")
                        nc.sync.dma_start(xt[:], xp[j, g])
                    for dd in range(GD):
                        d = g * GD + dd
                        xs = xt[:, dd * 512:(dd + 1) * 512]
                        st = (d == 0)
                        sp_ = (d == DCH - 1)
                        qb = d * 512
                        for m in range(QH):
                            nc.tensor.matmul(
                                accs[m][:], wq_sb[:, qb + m * 128:qb + (m + 1) * 128],
                                xs, start=st, stop=sp_)
                        kb = d * 256
                        nc.tensor.matmul(accs[4][:], wkv_sb[:, kb:kb + 128],
                                         xs, start=st, stop=sp_)
                        nc.tensor.matmul(accs[5][:], wkv_sb[:, kb + 128:kb + 256],
                                         xs, start=st, stop=sp_)

                # RoPE for q tiles and k tile
                for m in range(5):
                    dst = qrot[m][:, sq] if m < QH else krot[:, sq]
                    q_sb = stage.tile([128, 512], bf, tag="q_sb")
                    nc.scalar.copy(q_sb[:], accs[m][:])
                    rot_ps = ropep.tile([128, 512], f32, tag="rope_ps")
                    nc.tensor.matmul(rot_ps[:], rmat[:], q_sb[:],
                                     start=True, stop=True)
                    t1 = stage.tile([128, 512], f32, tag="t1")
                    nc.vector.tensor_mul(t1[:], accs[m][:], cosT[:, sq])
                    t2 = stage.tile([128, 512], f32, tag="t2")
                    nc.vector.tensor_mul(t2[:], rot_ps[:], sinT[:, sq])
                    nc.vector.tensor_add(dst, t1[:], t2[:])
                # region-scaled K columns (keys < 640 only)
                if j == 0:
                    nc.vector.tensor_mul(krot_s[:, 0:512], krot[:, 0:512],
                                         ksB[:, 0:512])
                elif j == 1:
                    nc.vector.tensor_mul(krot_s[:, 512:640], krot[:, 512:640],
                                         ksB[:, 512:640])

                # V: copy to SBUF, transpose 128x128 blocks into vnat
                v_sb = stage.tile([128, 512], bf, tag="v_sb")
                nc.scalar.copy(v_sb[:], accs[5][:])
                vt_ps = vtp.tile([128, 512], bf, tag="vt_ps")
                for b in range(4):
                    nc.tensor.transpose(vt_ps[:, b * 128:(b + 1) * 128],
                                        v_sb[:, b * 128:(b + 1) * 128], ident[:])
                nc.vector.tensor_copy(
                    vnat[:, (4 * j) * 128:(4 * j + 4) * 128], vt_ps[:])

        # ------------- Phase B+C: attention fused with o_proj --------------
        # Software-pipelined emission: the PE queue is in-order, so the
        # scores matmul of tile t+1 is EMITTED before the dn/av matmuls of
        # tile t (which wait on exp(t) on ACT). o_proj chunks of sq-block
        # j-1 (pure PE work, no ACT dependency) are interleaved into the
        # attention stream of block j so PE never idles on the exp/mask
        # chain. Mostly-masked diagonal tiles are trimmed to their live
        # column range (delta*128..512). Finalize (normalize) of a head is
        # emitted one tile into the next head; the [1,512]->[128,512]
        # reciprocal broadcast rides a DRAM round-trip DMA (no PE/PSUM).
        with tc.tile_pool(name="att_sb", bufs=3) as att_sb, \
             tc.tile_pool(name="sp", bufs=2, space="PSUM") as sp, \
             tc.tile_pool(name="avp", bufs=2, space="PSUM") as avp, \
             tc.tile_pool(name="dnp", bufs=2, space="PSUM") as dnp, \
             tc.tile_pool(name="op", bufs=2, space="PSUM") as op, \
             tc.tile_pool(name="nrm", bufs=2) as nrm, \
             tc.tile_pool(name="ost", bufs=2) as ost:
            # interleave plan: B items of block j + C chunks of block j-1
            plan = []
            for j in range(J):
                bi = [("B", j, h, i)
                      for h in range(QH) for i in range(4 * j + 4)]
                cc = [("C", j - 1, c) for c in range(DCH)] if j > 0 else []
                merged = []
                nb, ncc = len(bi), len(cc)
                ci = 0
                for idx, el in enumerate(bi):
                    merged.append(el)
                    # hold C back a few items so attn[3] finalize lands
                    while (ci < ncc
                           and idx >= 3
                           and ci + 1 <= (idx - 2) * ncc / (nb - 3)):
                        merged.append(cc[ci])
                        ci += 1
                merged.extend(cc[ci:])
                plan.extend(merged)
            plan.extend(("C", J - 1, c) for c in range(DCH))
            b_seq = [el for el in plan if el[0] == "B"]
            b_next = {}
            for a, b in zip(b_seq, b_seq[1:]):
                b_next[a] = b

            def emit_scores(el):
                _, j, h, i = el
                delta = i - 4 * j
                c0 = 128 * delta if delta >= 1 else 0   # trimmed live cols
                sq0 = j * 512
                s_ps = sp.tile([128, 512], f32, tag="s")
                # scaled K for full-region rows (j>=2), keys<640
                if j >= 2 and i < 5:
                    kt = krot_s[:, i * 128:(i + 1) * 128]
                else:
                    kt = krot[:, i * 128:(i + 1) * 128]
                nc.tensor.matmul(s_ps[:, c0:512], kt,
                                 qrot[h][:, sq0 + c0:sq0 + 512],
                                 start=True, stop=True)
                if j == 1 and i < 5:
                    # partial region rows: queries 611.. are cols 99..
                    nc.vector.tensor_scalar_mul(
                        s_ps[:, 99:512], s_ps[:, 99:512], ksT[:, i:i + 1])
                return s_ps

            state = {}

            def emit_tail(el, s_ps):
                _, j, h, i = el
                ni = 4 * j + 4
                delta = i - 4 * j
                c0 = 128 * delta if delta >= 1 else 0
                if i == 0:
                    state[(j, h)] = (
                        avp.tile([128, 512], f32, tag="av", name="acc_av"),
                        dnp.tile([1, 512], f32, tag="dn", name="acc_dn"))
                acc_av, acc_dn = state[(j, h)]
                e_sb = att_sb.tile([128, 512], bf, tag="e")
                nc.scalar.activation(e_sb[:, c0:512], s_ps[:, c0:512], EXP)
                if delta >= 0:
                    nc.vector.tensor_mul(
                        e_sb[:, c0:512], e_sb[:, c0:512],
                        masks[:, delta * 512 + c0:(delta + 1) * 512])
                st = (i == 0)
                sp_l = (i == ni - 1)
                eb = e_sb[:, c0:512]
                nc.tensor.matmul(acc_dn[:, c0:512], ones_col[:], eb,
                                 start=st, stop=sp_l)
                nc.tensor.matmul(acc_av[:, c0:512],
                                 vnat[:, i * 128:(i + 1) * 128],
                                 eb, start=st, stop=sp_l)

            def emit_finalize(j, h):
                sq = slice(j * 512, (j + 1) * 512)
                acc_av, acc_dn = state.pop((j, h))
                # 1/x = exp(-ln(x)) on ACT; partition-broadcast [1,512] ->
                # [128,512] via a DRAM round-trip (0-stride DRAM source AP)
                # so no PE matmul / PSUM bank is needed.
                lrec = nrm.tile([1, 512], f32, tag="lrec")
                nc.scalar.activation(lrec[:], acc_dn[:],
                                     mybir.ActivationFunctionType.Ln)
                rec2 = nrm.tile([1, 512], f32, tag="rec2")
                nc.scalar.activation(rec2[:], lrec[:], EXP, scale=-1.0)
                hg = j * QH + h
                nc.sync.dma_start(rscr[hg:hg + 1, :], rec2[:])
                rb_sb = nrm.tile([128, 512], f32, tag="rb_sb")
                nc.sync.dma_start(rb_sb[:],
                                  rscr[hg:hg + 1, :].broadcast_to((128, 512)))
                nc.vector.tensor_mul(attn[h][:, sq], acc_av[:], rb_sb[:])

            osb_state = {}

            def emit_cchunk(jc, c):
                sq = slice(jc * 512, (jc + 1) * 512)
                if c % 4 == 0:
                    osb_state[jc] = ost.tile([128, 4 * 512], bf, tag="o_sb",
                                             name="o_sb")
                o_sb = osb_state[jc]
                cb = c % 4
                o_ps = op.tile([128, 512], f32, tag="o")
                for h in range(QH):
                    nc.tensor.matmul(
                        o_ps[:],
                        wo_sb[:, h * D + c * 128:h * D + (c + 1) * 128],
                        attn[h][:, sq], start=(h == 0), stop=(h == QH - 1))
                if c % 2 == 0:
                    nc.scalar.copy(o_sb[:, cb * 512:(cb + 1) * 512], o_ps[:])
                else:
                    nc.vector.tensor_copy(
                        o_sb[:, cb * 512:(cb + 1) * 512], o_ps[:])
                if cb == 3:
                    nc.sync.dma_start(out[jc, c // 4], o_sb[:])

            s2_cur = emit_scores(b_seq[0])
            pending = None
            for el in plan:
                if el[0] == "B":
                    nxt = b_next.get(el)
                    s2_next = emit_scores(nxt) if nxt is not None else None
                    emit_tail(el, s2_cur)
                    s2_cur = s2_next
                    if pending is not None and pending != (el[1], el[2]):
                        emit_finalize(*pending)
                        pending = None
                    if el[3] == 4 * el[1] + 3:    # last key tile of (j, h)
                        pending = (el[1], el[2])
                else:
                    # a C chunk reads all four attn heads of its block: any
                    # still-pending finalize for that block must land first
                    if pending is not None and pending[0] == el[1]:
                        emit_finalize(*pending)
                        pending = None
                    emit_cchunk(el[1], el[2])
            if pending is not None:
                emit_finalize(*pending)

    # Split multi-wait instructions onto standalone EventSemaphore insts.
    import bass_rust
    bass_rust.generate_event_semaphores(nc)
    return nc


def _get_compiled():
    if "nc" not in _CACHE:
        _CACHE["nc"] = _build_bass()
        _CACHE["const"] = _host_constants()
    return _CACHE["nc"], _CACHE["const"]


def kernel(hidden_states, wq, wk, wv, wo, _trace=False):
    from concourse.bass_utils import run_bass_kernel_spmd

    nc, cst = _get_compiled()
    bf16 = _bf16()

    x = np.asarray(hidden_states, dtype=np.float32).reshape(S, D)
    xT = x.T                                             # [D, S]
    # xp[j, g, p, dd*512+f] = xT[128*(8g+dd)+p, 512j+f]
    t = xT.reshape(G, GD, 128, J, 512)                   # [g, dd, p, j, f]
    xpk = np.ascontiguousarray(
        t.transpose(3, 0, 2, 1, 4).reshape(J, G, 128, GD * 512)).astype(bf16)
    wq = np.asarray(wq, dtype=np.float32)
    wk = np.asarray(wk, dtype=np.float32)
    wv = np.asarray(wv, dtype=np.float32)
    wo = np.asarray(wo, dtype=np.float32)
    scale = 1.0 / math.sqrt(HD)

    in_maps = []
    for d in range(NCORES):
        wq_d = wq[d * QH * HD:(d + 1) * QH * HD] * scale      # [512, D]
        wqT = wq_d.T                                          # [4096, 512]
        wqpk = np.ascontiguousarray(
            wqT.reshape(G, GD, 128, 512).transpose(0, 2, 1, 3)
            .reshape(G, 128, GD * 512)).astype(bf16)
        wkT = wk[d * HD:(d + 1) * HD].T.reshape(G, GD, 128, 128)
        wvT = wv[d * HD:(d + 1) * HD].T.reshape(G, GD, 128, 128)
        kv = np.concatenate([wkT, wvT], axis=-1)              # [G, GD, 128, 256]
        wkvpk = np.ascontiguousarray(
            kv.transpose(0, 2, 1, 3).reshape(G, 128, GD * 256)).astype(bf16)
        woT = wo[:, d * QH * HD:(d + 1) * QH * HD].T          # [512, 4096]
        wopk = np.ascontiguousarray(
            woT.reshape(QH, 128, D).transpose(1, 0, 2)
            .reshape(128, QH * D)).astype(bf16)
        in_maps.append({
            "xp": xpk,
            "wqp": wqpk,
            "wkvp": wkvpk,
            "wop": wopk,
            "cosT": cst["cosT"], "sinT": cst["sinT"],
            "rmat": cst["rmat"], "ident": cst["ident"],
            "masks": cst["masks"], "ksT": cst["ksT"], "ksB": cst["ksB"],
            "ones_col": cst["ones_col"],
        })

    res = run_bass_kernel_spmd(nc, in_maps, core_ids=list(range(NCORES)),
                               trace=_trace)
    acc = res.results[0]["out"].astype(np.float32)
    for d in range(1, NCORES):
        acc += res.results[d]["out"].astype(np.float32)
    # out_t[j, cq, p, cb*512+f] = out.T[128*(4cq+cb)+p, 512j+f]
    acc = acc.reshape(J, DCH // 4, 128, 4, 512)          # [j, cq, p, cb, f]
    outp = acc.transpose(0, 4, 1, 3, 2).reshape(S, D)    # [512j+f, 128(4cq+cb)+p]
    outp = outp.reshape(1, S, D).astype(np.float32)
    if _trace:
        _CACHE["last_results"] = res
    return outp


# revision 22
# speedup vs baseline: 1.4774x; 1.0183x over previous
"""Trainium2 Bass kernel for nn_AttnAdapter: GQA attention with RoPE,
region-based enhance/suppress score scaling, causal mask, o_proj.

Sharding: tensor-parallel over heads across 8 NeuronCores. Core d holds
q-heads 4d..4d+3 (wq rows), kv-head d (wk/wv rows), and wo columns
512d..512(d+1). Each core computes a full [S, D] partial of the output;
the host sums the 8 partials (the TP all-reduce, done at unshard time).

v2: all matmuls in bf16 (fp32 PSUM accumulate), weights resident in
SBUF (single load), exp batched over [128,1024] PSUM pairs, region
scale folded into pre-scaled K columns, reciprocal on DVE, all PSUM
evictions on DVE, batched DMAs.
"""

import math

import numpy as np

# ---- problem constants (hardcoded; kernel.py must be self-contained) ----
S = 2048          # sequence length
D = 4096          # model dim
HD = 128          # head dim
NCORES = 8
QH = 4            # q heads per core
SYS_LEN, IMG_LEN = 35, 576
BOUND = SYS_LEN + IMG_LEN          # 611
ENH, SUP = 1.5, 0.5
ROPE_BASE = 10000.0

J = 4             # sq tiles of 512
NSK = 16          # sk tiles of 128
DCH = 32          # D chunks of 128
G = 4             # d-groups of 8 chunks
GD = 8            # d-chunks per group

_CACHE = {}


def _bf16():
    import ml_dtypes
    return ml_dtypes.bfloat16


def _host_constants():
    bf16 = _bf16()
    inv_freq = 1.0 / (ROPE_BASE ** (np.arange(0, HD, 2, dtype=np.float32) / HD))
    pos = np.arange(S, dtype=np.float32)
    freqs = pos[:, None] * inv_freq[None, :]              # [S, 64]
    emb = np.concatenate([freqs, freqs], axis=-1)         # [S, 128]
    cosT = np.ascontiguousarray(np.cos(emb).T.astype(np.float32))  # [128, S]
    sinT = np.ascontiguousarray(np.sin(emb).T.astype(np.float32))

    # rotate_half as a matmul: rot = R @ q (in [hd, s] layout).
    # matmul(out, lhsT, rhs) = lhsT.T @ rhs, so feed RT = R.T.
    RT = np.zeros((HD, HD), dtype=np.float32)
    half = HD // 2
    for c in range(half):
        RT[c + half, c] = -1.0      # rot[c] = -q[c+64]
    for c in range(half, HD):
        RT[c - half, c] = 1.0       # rot[c] = q[c-64]

    ident = np.eye(HD, dtype=np.float32)

    # Diagonal-tile causal masks, T layout [sk 128, sq 512]:
    # tile (i=4j+delta, j): valid (keep) iff sq >= sk  <=>  f >= 128*delta + p
    masks = np.zeros((HD, 4 * 512), dtype=np.float32)
    p = np.arange(128)[:, None]
    f = np.arange(512)[None, :]
    for delta in range(4):
        masks[:, delta * 512:(delta + 1) * 512] = (f >= 128 * delta + p)

    kpos = np.arange(S)
    key_scale = np.where(kpos < SYS_LEN, SUP,
                         np.where(kpos < BOUND, ENH, 1.0)).astype(np.float32)
    # per-partition scale per sk-tile (j=1 partial-row DVE path)
    ksT = np.ascontiguousarray(key_scale[:5 * 128].reshape(5, 128).T)  # [128, 5]
    # per-key-column broadcast (for pre-scaling krot columns, j>=2 path)
    ksB = np.ascontiguousarray(
        np.broadcast_to(key_scale[:640][None, :], (HD, 640)))          # [128, 640]

    return dict(
        cosT=cosT, sinT=sinT,
        rmat=RT.astype(bf16), ident=ident.astype(bf16),
        masks=masks.astype(bf16), ksT=ksT, ksB=ksB.astype(bf16),
        ones_col=np.ones((HD, 1), dtype=bf16),
    )


def _build_bass():
    import concourse.bass as bass
    import concourse.mybir as mybir
    from concourse.tile import TileContext
    from contextlib import ExitStack

    f32 = mybir.dt.float32
    f32r = mybir.dt.float32r
    bf = mybir.dt.bfloat16

    nc = bass.Bass()
    # x packed per (j, g): [128, GD*512] with [p, dd*512+f] = x.T[128(8g+dd)+p, 512j+f]
    xp = nc.dram_tensor("xp", [J, G, 128, GD * 512], bf, kind="ExternalInput")
    # wq packed per g: [128, GD*512] with [p, dd*512+f] = wq_d.T[128(8g+dd)+p, f]
    wqp = nc.dram_tensor("wqp", [G, 128, GD * 512], bf, kind="ExternalInput")
    # wk/wv packed per g: [128, GD*256], cols dd*256+0:128 = wkT, 128:256 = wvT
    wkvp = nc.dram_tensor("wkvp", [G, 128, GD * 256], bf, kind="ExternalInput")
    # wo packed: [128, 4*4096] with [p, h*4096+f] = wo[:, core].T[128h+p, f]
    wop = nc.dram_tensor("wop", [128, QH * D], bf, kind="ExternalInput")
    cosT_d = nc.dram_tensor("cosT", [HD, S], f32, kind="ExternalInput")
    sinT_d = nc.dram_tensor("sinT", [HD, S], f32, kind="ExternalInput")
    rmat_d = nc.dram_tensor("rmat", [HD, HD], bf, kind="ExternalInput")
    ident_d = nc.dram_tensor("ident", [HD, HD], bf, kind="ExternalInput")
    masks_d = nc.dram_tensor("masks", [HD, 4 * 512], bf, kind="ExternalInput")
    ksT_d = nc.dram_tensor("ksT", [HD, 5], f32, kind="ExternalInput")
    ksB_d = nc.dram_tensor("ksB", [HD, 640], bf, kind="ExternalInput")
    onesc_d = nc.dram_tensor("ones_col", [HD, 1], bf, kind="ExternalInput")
    # per-(j,h) softmax 1/denom staging for the DMA partition-broadcast
    rscr = nc.dram_tensor("rscr", [J * QH, 512], f32, kind="Internal")
    # out.T tiles: out_t[j, cq, p, cb*512+f] = out.T[128*(4cq+cb)+p, 512j+f]
    out = nc.dram_tensor("out", [J, DCH // 4, 128, 4 * 512], bf,
                         kind="ExternalOutput")

    EXP = mybir.ActivationFunctionType.Exp

    with TileContext(nc) as tc, ExitStack() as ctx:
        # ---- resident weights (single load, used all phases) ----
        # Order matters for startup latency: the first matmuls need only
        # wq/wkv chunk g=0 and x(j=0,g=0); everything else trails behind.
        wsb = ctx.enter_context(tc.tile_pool(name="wsb", bufs=1))
        wq_sb = wsb.tile([128, DCH * 512], bf)
        wkv_sb = wsb.tile([128, DCH * 256], bf)
        nc.sync.dma_start(wq_sb[:, 0:GD * 512], wqp[0])
        nc.sync.dma_start(wkv_sb[:, 0:GD * 256], wkvp[0])
        xgp = ctx.enter_context(tc.tile_pool(name="xg", bufs=3))
        xpre = {}
        xt0 = xgp.tile([128, GD * 512], bf, tag="xt", name="xt0")
        nc.sync.dma_start(xt0[:], xp[0, 0])
        xpre[(0, 0)] = xt0
        for g in range(1, G):
            nc.sync.dma_start(wq_sb[:, g * GD * 512:(g + 1) * GD * 512], wqp[g])
            nc.sync.dma_start(wkv_sb[:, g * GD * 256:(g + 1) * GD * 256], wkvp[g])
            if g <= 2:
                xt_n = xgp.tile([128, GD * 512], bf, tag="xt", name=f"xt{g}")
                nc.sync.dma_start(xt_n[:], xp[0, g])
                xpre[(0, g)] = xt_n

        # const tiles allocated here; their DMAs are deferred until after
        # j=0's x tiles are queued so phase A's first block never starves
        const = ctx.enter_context(tc.tile_pool(name="const", bufs=1))
        cosT = const.tile([HD, S], f32)
        sinT = const.tile([HD, S], f32)
        rmat = const.tile([HD, HD], bf)
        ident = const.tile([HD, HD], bf)
        masks = const.tile([HD, 4 * 512], bf)
        ksT = const.tile([HD, 5], f32)
        ksB = const.tile([HD, 640], bf)
        ones_col = const.tile([HD, 1], bf)
        wo_sb = const.tile([128, QH * D], bf)

        def emit_const_dmas():
            nc.sync.dma_start(cosT[:], cosT_d[:, :])
            nc.sync.dma_start(sinT[:], sinT_d[:, :])
            nc.sync.dma_start(rmat[:], rmat_d[:, :])
            nc.sync.dma_start(ident[:], ident_d[:, :])
            nc.sync.dma_start(masks[:], masks_d[:, :])
            nc.sync.dma_start(ksT[:], ksT_d[:, :])
            nc.sync.dma_start(ksB[:], ksB_d[:, :])
            nc.sync.dma_start(ones_col[:], onesc_d[:, :])
            nc.sync.dma_start(wo_sb[:], wop[:, :])

        persist = ctx.enter_context(tc.tile_pool(name="persist", bufs=1))
        qrot = [persist.tile([HD, S], bf, name=f"qrot{m}") for m in range(QH)]
        krot = persist.tile([HD, S], bf)
        krot_s = persist.tile([HD, 640], bf)     # region-scaled K cols (keys<640)
        vnat = persist.tile([HD, NSK * HD], bf)  # tile i at cols i*128
        attn = [persist.tile([HD, S], bf, name=f"attn{h}") for h in range(QH)]

        # ---------------- Phase A: projections + RoPE + V transpose --------
        with tc.tile_pool(name="accp", bufs=1, space="PSUM") as accp, \
             tc.tile_pool(name="ropep", bufs=1, space="PSUM") as ropep, \
             tc.tile_pool(name="vtp", bufs=1, space="PSUM") as vtp, \
             tc.tile_pool(name="stage", bufs=2) as stage:
            for j in range(J):
                sq = slice(j * 512, (j + 1) * 512)
                accs = [accp.tile([128, 512], f32, name=f"acc{m}") for m in range(6)]
                for g in range(G):
                    if (j, g) in xpre:
                        xt = xpre.pop((j, g))
                    else:
                        xt = xgp.tile([128, GD * 512], bf, tag="xt")
                        nc.sync.dma_start(xt[:], xp[j, g])
                        if j == 0 and g == G - 1:
                            emit_const_dmas()
                    for dd in range(GD):
                        d = g * GD + dd
                        xs = xt[:, dd * 512:(dd + 1) * 512]
                        st = (d == 0)
                        sp_ = (d == DCH - 1)
                        qb = d * 512
                        for m in range(QH):
                            nc.tensor.matmul(
                                accs[m][:], wq_sb[:, qb + m * 128:qb + (m + 1) * 128],
                                xs, start=st, stop=sp_)
                        kb = d * 256
                        nc.tensor.matmul(accs[4][:], wkv_sb[:, kb:kb + 128],
                                         xs, start=st, stop=sp_)
                        nc.tensor.matmul(accs[5][:], wkv_sb[:, kb + 128:kb + 256],
                                         xs, start=st, stop=sp_)

                # RoPE for q tiles and k tile
                for m in range(5):
                    dst = qrot[m][:, sq] if m < QH else krot[:, sq]
                    q_sb = stage.tile([128, 512], bf, tag="q_sb")
                    nc.scalar.copy(q_sb[:], accs[m][:])
                    rot_ps = ropep.tile([128, 512], f32, tag="rope_ps")
                    nc.tensor.matmul(rot_ps[:], rmat[:], q_sb[:],
                                     start=True, stop=True)
                    t1 = stage.tile([128, 512], f32, tag="t1")
                    nc.vector.tensor_mul(t1[:], accs[m][:], cosT[:, sq])
                    t2 = stage.tile([128, 512], f32, tag="t2")
                    nc.vector.tensor_mul(t2[:], rot_ps[:], sinT[:, sq])
                    nc.vector.tensor_add(dst, t1[:], t2[:])
                # region-scaled K columns (keys < 640 only)
                if j == 0:
                    nc.vector.tensor_mul(krot_s[:, 0:512], krot[:, 0:512],
                                         ksB[:, 0:512])
                elif j == 1:
                    nc.vector.tensor_mul(krot_s[:, 512:640], krot[:, 512:640],
                                         ksB[:, 512:640])

                # V: copy to SBUF, transpose 128x128 blocks into vnat
                v_sb = stage.tile([128, 512], bf, tag="v_sb")
                nc.scalar.copy(v_sb[:], accs[5][:])
                vt_ps = vtp.tile([128, 512], bf, tag="vt_ps")
                for b in range(4):
                    nc.tensor.transpose(vt_ps[:, b * 128:(b + 1) * 128],
                                        v_sb[:, b * 128:(b + 1) * 128], ident[:])
                nc.vector.tensor_copy(
                    vnat[:, (4 * j) * 128:(4 * j + 4) * 128], vt_ps[:])

        # ------------- Phase B+C: attention fused with o_proj --------------
        # Software-pipelined emission: the PE queue is in-order, so the
        # scores matmul of tile t+1 is EMITTED before the dn/av matmuls of
        # tile t (which wait on exp(t) on ACT). o_proj chunks of sq-block
        # j-1 (pure PE work, no ACT dependency) are interleaved into the
        # attention stream of block j so PE never idles on the exp/mask
        # chain. Mostly-masked diagonal tiles are trimmed to their live
        # column range (delta*128..512). Finalize (normalize) of a head is
        # emitted one tile into the next head; the [1,512]->[128,512]
        # reciprocal broadcast rides a DRAM round-trip DMA (no PE/PSUM).
        with tc.tile_pool(name="att_sb", bufs=3) as att_sb, \
             tc.tile_pool(name="sp", bufs=2, space="PSUM") as sp, \
             tc.tile_pool(name="avp", bufs=2, space="PSUM") as avp, \
             tc.tile_pool(name="dnp", bufs=2, space="PSUM") as dnp, \
             tc.tile_pool(name="op", bufs=2, space="PSUM") as op, \
             tc.tile_pool(name="nrm", bufs=2) as nrm, \
             tc.tile_pool(name="ost", bufs=2) as ost:
            # interleave plan: B items of block j + C chunks of block j-1
            plan = []
            # j=3 processes head 0 last so the trailing o_proj chunks (whose
            # matmuls end with head 0) wait minimally on its finalize
            horder = {j: list(range(QH)) for j in range(J)}
            horder[J - 1] = [1, 2, 3, 0]
            for j in range(J):
                bi = [("B", j, h, i)
                      for h in horder[j] for i in range(4 * j + 4)]
                cc = [("C", j - 1, c) for c in range(DCH)] if j > 0 else []
                merged = []
                nb, ncc = len(bi), len(cc)
                ci = 0
                for idx, el in enumerate(bi):
                    merged.append(el)
                    # hold C back a few items so attn[3] finalize lands
                    while (ci < ncc
                           and idx >= 3
                           and ci + 1 <= (idx - 2) * ncc / (nb - 3)):
                        merged.append(cc[ci])
                        ci += 1
                merged.extend(cc[ci:])
                plan.extend(merged)
            plan.extend(("C", J - 1, c) for c in range(DCH))
            b_seq = [el for el in plan if el[0] == "B"]
            b_next = {}
            for a, b in zip(b_seq, b_seq[1:]):
                b_next[a] = b

            def emit_scores(el):
                _, j, h, i = el
                delta = i - 4 * j
                c0 = 128 * delta if delta >= 1 else 0   # trimmed live cols
                sq0 = j * 512
                s_ps = sp.tile([128, 512], f32, tag="s")
                # scaled K for full-region rows (j>=2), keys<640
                if j >= 2 and i < 5:
                    kt = krot_s[:, i * 128:(i + 1) * 128]
                else:
                    kt = krot[:, i * 128:(i + 1) * 128]
                nc.tensor.matmul(s_ps[:, c0:512], kt,
                                 qrot[h][:, sq0 + c0:sq0 + 512],
                                 start=True, stop=True)
                if j == 1 and i < 5:
                    # partial region rows: queries 611.. are cols 99..
                    nc.vector.tensor_scalar_mul(
                        s_ps[:, 99:512], s_ps[:, 99:512], ksT[:, i:i + 1])
                return s_ps

            state = {}

            def emit_tail(el, s_ps):
                _, j, h, i = el
                ni = 4 * j + 4
                delta = i - 4 * j
                c0 = 128 * delta if delta >= 1 else 0
                if i == 0:
                    state[(j, h)] = (
                        avp.tile([128, 512], f32, tag="av", name="acc_av"),
                        dnp.tile([1, 512], f32, tag="dn", name="acc_dn"))
                acc_av, acc_dn = state[(j, h)]
                e_sb = att_sb.tile([128, 512], bf, tag="e")
                nc.scalar.activation(e_sb[:, c0:512], s_ps[:, c0:512], EXP)
                if delta >= 0:
                    nc.vector.tensor_mul(
                        e_sb[:, c0:512], e_sb[:, c0:512],
                        masks[:, delta * 512 + c0:(delta + 1) * 512])
                st = (i == 0)
                sp_l = (i == ni - 1)
                eb = e_sb[:, c0:512]
                nc.tensor.matmul(acc_dn[:, c0:512], ones_col[:], eb,
                                 start=st, stop=sp_l)
                nc.tensor.matmul(acc_av[:, c0:512],
                                 vnat[:, i * 128:(i + 1) * 128],
                                 eb, start=st, stop=sp_l)

            def emit_finalize(j, h):
                sq = slice(j * 512, (j + 1) * 512)
                acc_av, acc_dn = state.pop((j, h))
                # 1/x = exp(-ln(x)) on ACT; partition-broadcast [1,512] ->
                # [128,512] via a DRAM round-trip (0-stride DRAM source AP)
                # so no PE matmul / PSUM bank is needed.
                lrec = nrm.tile([1, 512], f32, tag="lrec")
                nc.scalar.activation(lrec[:], acc_dn[:],
                                     mybir.ActivationFunctionType.Ln)
                rec2 = nrm.tile([1, 512], f32, tag="rec2")
                nc.scalar.activation(rec2[:], lrec[:], EXP, scale=-1.0)
                hg = j * QH + h
                nc.sync.dma_start(rscr[hg:hg + 1, :], rec2[:])
                rb_sb = nrm.tile([128, 512], f32, tag="rb_sb")
                nc.sync.dma_start(rb_sb[:],
                                  rscr[hg:hg + 1, :].broadcast_to((128, 512)))
                nc.vector.tensor_mul(attn[h][:, sq], acc_av[:], rb_sb[:])

            osb_state = {}

            def emit_cchunk(jc, c):
                sq = slice(jc * 512, (jc + 1) * 512)
                if c % 4 == 0:
                    osb_state[jc] = ost.tile([128, 4 * 512], bf, tag="o_sb",
                                             name="o_sb")
                o_sb = osb_state[jc]
                cb = c % 4
                o_ps = op.tile([128, 512], f32, tag="o")
                for hi, h in enumerate([1, 2, 3, 0]):
                    nc.tensor.matmul(
                        o_ps[:],
                        wo_sb[:, h * D + c * 128:h * D + (c + 1) * 128],
                        attn[h][:, sq], start=(hi == 0), stop=(hi == QH - 1))
                if c % 2 == 0:
                    nc.scalar.copy(o_sb[:, cb * 512:(cb + 1) * 512], o_ps[:])
                else:
                    nc.vector.tensor_copy(
                        o_sb[:, cb * 512:(cb + 1) * 512], o_ps[:])
                if cb == 3:
                    nc.sync.dma_start(out[jc, c // 4], o_sb[:])

            s2_cur = emit_scores(b_seq[0])
            pending = None
            for el in plan:
                if el[0] == "B":
                    nxt = b_next.get(el)
                    s2_next = emit_scores(nxt) if nxt is not None else None
                    emit_tail(el, s2_cur)
                    s2_cur = s2_next
                    if pending is not None and pending != (el[1], el[2]):
                        emit_finalize(*pending)
                        pending = None
                    if el[3] == 4 * el[1] + 3:    # last key tile of (j, h)
                        pending = (el[1], el[2])
                else:
                    # a C chunk reads all four attn heads of its block: any
                    # still-pending finalize for that block must land first
                    if pending is not None and pending[0] == el[1]:
                        emit_finalize(*pending)
                        pending = None
                    emit_cchunk(el[1], el[2])
            if pending is not None:
                emit_finalize(*pending)

    # Split multi-wait instructions onto standalone EventSemaphore insts.
    import bass_rust
    bass_rust.generate_event_semaphores(nc)
    return nc


def _get_compiled():
    if "nc" not in _CACHE:
        _CACHE["nc"] = _build_bass()
        _CACHE["const"] = _host_constants()
    return _CACHE["nc"], _CACHE["const"]


def kernel(hidden_states, wq, wk, wv, wo, _trace=False):
    from concourse.bass_utils import run_bass_kernel_spmd

    nc, cst = _get_compiled()
    bf16 = _bf16()

    x = np.asarray(hidden_states, dtype=np.float32).reshape(S, D)
    xT = x.T                                             # [D, S]
    # xp[j, g, p, dd*512+f] = xT[128*(8g+dd)+p, 512j+f]
    t = xT.reshape(G, GD, 128, J, 512)                   # [g, dd, p, j, f]
    xpk = np.ascontiguousarray(
        t.transpose(3, 0, 2, 1, 4).reshape(J, G, 128, GD * 512)).astype(bf16)
    wq = np.asarray(wq, dtype=np.float32)
    wk = np.asarray(wk, dtype=np.float32)
    wv = np.asarray(wv, dtype=np.float32)
    wo = np.asarray(wo, dtype=np.float32)
    scale = 1.0 / math.sqrt(HD)

    in_maps = []
    for d in range(NCORES):
        wq_d = wq[d * QH * HD:(d + 1) * QH * HD] * scale      # [512, D]
        wqT = wq_d.T                                          # [4096, 512]
        wqpk = np.ascontiguousarray(
            wqT.reshape(G, GD, 128, 512).transpose(0, 2, 1, 3)
            .reshape(G, 128, GD * 512)).astype(bf16)
        wkT = wk[d * HD:(d + 1) * HD].T.reshape(G, GD, 128, 128)
        wvT = wv[d * HD:(d + 1) * HD].T.reshape(G, GD, 128, 128)
        kv = np.concatenate([wkT, wvT], axis=-1)              # [G, GD, 128, 256]
        wkvpk = np.ascontiguousarray(
            kv.transpose(0, 2, 1, 3).reshape(G, 128, GD * 256)).astype(bf16)
        woT = wo[:, d * QH * HD:(d + 1) * QH * HD].T          # [512, 4096]
        wopk = np.ascontiguousarray(
            woT.reshape(QH, 128, D).transpose(1, 0, 2)
            .reshape(128, QH * D)).astype(bf16)
        in_maps.append({
            "xp": xpk,
            "wqp": wqpk,
            "wkvp": wkvpk,
            "wop": wopk,
            "cosT": cst["cosT"], "sinT": cst["sinT"],
            "rmat": cst["rmat"], "ident": cst["ident"],
            "masks": cst["masks"], "ksT": cst["ksT"], "ksB": cst["ksB"],
            "ones_col": cst["ones_col"],
        })

    res = run_bass_kernel_spmd(nc, in_maps, core_ids=list(range(NCORES)),
                               trace=_trace)
    acc = res.results[0]["out"].astype(np.float32)
    for d in range(1, NCORES):
        acc += res.results[d]["out"].astype(np.float32)
    # out_t[j, cq, p, cb*512+f] = out.T[128*(4cq+cb)+p, 512j+f]
    acc = acc.reshape(J, DCH // 4, 128, 4, 512)          # [j, cq, p, cb, f]
    outp = acc.transpose(0, 4, 1, 3, 2).reshape(S, D)    # [512j+f, 128(4cq+cb)+p]
    outp = outp.reshape(1, S, D).astype(np.float32)
    if _trace:
        _CACHE["last_results"] = res
    return outp


# revision 24
# speedup vs baseline: 1.4986x; 1.0144x over previous
"""Trainium2 Bass kernel for nn_AttnAdapter: GQA attention with RoPE,
region-based enhance/suppress score scaling, causal mask, o_proj.

Sharding: tensor-parallel over heads across 8 NeuronCores. Core d holds
q-heads 4d..4d+3 (wq rows), kv-head d (wk/wv rows), and wo columns
512d..512(d+1). Each core computes a full [S, D] partial of the output;
the host sums the 8 partials (the TP all-reduce, done at unshard time).

v2: all matmuls in bf16 (fp32 PSUM accumulate), weights resident in
SBUF (single load), exp batched over [128,1024] PSUM pairs, region
scale folded into pre-scaled K columns, reciprocal on DVE, all PSUM
evictions on DVE, batched DMAs.
"""

import math

import numpy as np

# ---- problem constants (hardcoded; kernel.py must be self-contained) ----
S = 2048          # sequence length
D = 4096          # model dim
HD = 128          # head dim
NCORES = 8
QH = 4            # q heads per core
SYS_LEN, IMG_LEN = 35, 576
BOUND = SYS_LEN + IMG_LEN          # 611
ENH, SUP = 1.5, 0.5
ROPE_BASE = 10000.0

J = 4             # sq tiles of 512
NSK = 16          # sk tiles of 128
DCH = 32          # D chunks of 128
G = 4             # d-groups of 8 chunks
GD = 8            # d-chunks per group

_CACHE = {}


def _bf16():
    import ml_dtypes
    return ml_dtypes.bfloat16


def _host_constants():
    bf16 = _bf16()
    inv_freq = 1.0 / (ROPE_BASE ** (np.arange(0, HD, 2, dtype=np.float32) / HD))
    pos = np.arange(S, dtype=np.float32)
    freqs = pos[:, None] * inv_freq[None, :]              # [S, 64]
    emb = np.concatenate([freqs, freqs], axis=-1)         # [S, 128]
    cosT = np.ascontiguousarray(np.cos(emb).T.astype(np.float32))  # [128, S]
    sinT = np.ascontiguousarray(np.sin(emb).T.astype(np.float32))

    # rotate_half as a matmul: rot = R @ q (in [hd, s] layout).
    # matmul(out, lhsT, rhs) = lhsT.T @ rhs, so feed RT = R.T.
    RT = np.zeros((HD, HD), dtype=np.float32)
    half = HD // 2
    for c in range(half):
        RT[c + half, c] = -1.0      # rot[c] = -q[c+64]
    for c in range(half, HD):
        RT[c - half, c] = 1.0       # rot[c] = q[c-64]

    ident = np.eye(HD, dtype=np.float32)

    # Diagonal-tile causal masks, T layout [sk 128, sq 512]:
    # tile (i=4j+delta, j): valid (keep) iff sq >= sk  <=>  f >= 128*delta + p
    masks = np.zeros((HD, 4 * 512), dtype=np.float32)
    p = np.arange(128)[:, None]
    f = np.arange(512)[None, :]
    for delta in range(4):
        masks[:, delta * 512:(delta + 1) * 512] = (f >= 128 * delta + p)

    kpos = np.arange(S)
    key_scale = np.where(kpos < SYS_LEN, SUP,
                         np.where(kpos < BOUND, ENH, 1.0)).astype(np.float32)
    # per-partition scale per sk-tile (j=1 partial-row DVE path)
    ksT = np.ascontiguousarray(key_scale[:5 * 128].reshape(5, 128).T)  # [128, 5]
    # per-key-column broadcast (for pre-scaling krot columns, j>=2 path)
    ksB = np.ascontiguousarray(
        np.broadcast_to(key_scale[:640][None, :], (HD, 640)))          # [128, 640]

    return dict(
        cosT=cosT, sinT=sinT,
        rmat=RT.astype(bf16), ident=ident.astype(bf16),
        masks=masks.astype(bf16), ksT=ksT, ksB=ksB.astype(bf16),
        ones_col=np.ones((HD, 1), dtype=bf16),
    )


def _build_bass():
    import concourse.bass as bass
    import concourse.mybir as mybir
    from concourse.tile import TileContext
    from contextlib import ExitStack

    f32 = mybir.dt.float32
    f32r = mybir.dt.float32r
    bf = mybir.dt.bfloat16

    nc = bass.Bass()
    # x packed per (j, g): [128, GD*512] with [p, dd*512+f] = x.T[128(8g+dd)+p, 512j+f]
    xp = nc.dram_tensor("xp", [J, G, 128, GD * 512], bf, kind="ExternalInput")
    # wq packed per g: [128, GD*512] with [p, dd*512+f] = wq_d.T[128(8g+dd)+p, f]
    wqp = nc.dram_tensor("wqp", [G, 128, GD * 512], bf, kind="ExternalInput")
    # wk/wv packed per g: [128, GD*256], cols dd*256+0:128 = wkT, 128:256 = wvT
    wkvp = nc.dram_tensor("wkvp", [G, 128, GD * 256], bf, kind="ExternalInput")
    # wo packed: [128, 4*4096] with [p, h*4096+f] = wo[:, core].T[128h+p, f]
    wop = nc.dram_tensor("wop", [128, QH * D], bf, kind="ExternalInput")
    cosT_d = nc.dram_tensor("cosT", [HD, S], f32, kind="ExternalInput")
    sinT_d = nc.dram_tensor("sinT", [HD, S], f32, kind="ExternalInput")
    rmat_d = nc.dram_tensor("rmat", [HD, HD], bf, kind="ExternalInput")
    ident_d = nc.dram_tensor("ident", [HD, HD], bf, kind="ExternalInput")
    masks_d = nc.dram_tensor("masks", [HD, 4 * 512], bf, kind="ExternalInput")
    ksT_d = nc.dram_tensor("ksT", [HD, 5], f32, kind="ExternalInput")
    ksB_d = nc.dram_tensor("ksB", [HD, 640], bf, kind="ExternalInput")
    onesc_d = nc.dram_tensor("ones_col", [HD, 1], bf, kind="ExternalInput")
    # per-(j,h) softmax 1/denom staging for the DMA partition-broadcast
    rscr = nc.dram_tensor("rscr", [J * QH, 512], f32, kind="Internal")
    # out.T tiles: out_t[j, cq, p, cb*512+f] = out.T[128*(4cq+cb)+p, 512j+f]
    out = nc.dram_tensor("out", [J, DCH // 4, 128, 4 * 512], bf,
                         kind="ExternalOutput")

    EXP = mybir.ActivationFunctionType.Exp

    with TileContext(nc) as tc, ExitStack() as ctx:
        # ---- resident weights (single load, used all phases) ----
        # Order matters for startup latency: the first matmuls need only
        # wq/wkv chunk g=0 and x(j=0,g=0); everything else trails behind.
        wsb = ctx.enter_context(tc.tile_pool(name="wsb", bufs=1))
        wq_sb = wsb.tile([128, DCH * 512], bf)
        wkv_sb = wsb.tile([128, DCH * 256], bf)
        nc.sync.dma_start(wq_sb[:, 0:GD * 512], wqp[0])
        nc.sync.dma_start(wkv_sb[:, 0:GD * 256], wkvp[0])
        xgp = ctx.enter_context(tc.tile_pool(name="xg", bufs=3))
        xpre = {}
        xt0 = xgp.tile([128, GD * 512], bf, tag="xt", name="xt0")
        nc.sync.dma_start(xt0[:], xp[0, 0])
        xpre[(0, 0)] = xt0
        for g in range(1, G):
            nc.sync.dma_start(wq_sb[:, g * GD * 512:(g + 1) * GD * 512], wqp[g])
            nc.sync.dma_start(wkv_sb[:, g * GD * 256:(g + 1) * GD * 256], wkvp[g])
            if g <= 2:
                xt_n = xgp.tile([128, GD * 512], bf, tag="xt", name=f"xt{g}")
                nc.sync.dma_start(xt_n[:], xp[0, g])
                xpre[(0, g)] = xt_n

        # const tiles allocated here; their DMAs are deferred until after
        # j=0's x tiles are queued so phase A's first block never starves
        const = ctx.enter_context(tc.tile_pool(name="const", bufs=1))
        cosT = const.tile([HD, S], f32)
        sinT = const.tile([HD, S], f32)
        rmat = const.tile([HD, HD], bf)
        ident = const.tile([HD, HD], bf)
        masks = const.tile([HD, 4 * 512], bf)
        ksT = const.tile([HD, 5], f32)
        ksB = const.tile([HD, 640], bf)
        ones_col = const.tile([HD, 1], bf)
        wo_sb = const.tile([128, QH * D], bf)

        def emit_const_dmas():
            nc.sync.dma_start(cosT[:], cosT_d[:, :])
            nc.sync.dma_start(sinT[:], sinT_d[:, :])
            nc.sync.dma_start(rmat[:], rmat_d[:, :])
            nc.sync.dma_start(ident[:], ident_d[:, :])
            nc.sync.dma_start(masks[:], masks_d[:, :])
            nc.sync.dma_start(ksT[:], ksT_d[:, :])
            nc.sync.dma_start(ksB[:], ksB_d[:, :])
            nc.sync.dma_start(ones_col[:], onesc_d[:, :])
            nc.sync.dma_start(wo_sb[:], wop[:, :])

        persist = ctx.enter_context(tc.tile_pool(name="persist", bufs=1))
        qrot = [persist.tile([HD, S], bf, name=f"qrot{m}") for m in range(QH)]
        krot = persist.tile([HD, S], bf)
        krot_s = persist.tile([HD, 640], bf)     # region-scaled K cols (keys<640)
        vnat = persist.tile([HD, NSK * HD], bf)  # tile i at cols i*128
        attn = [persist.tile([HD, S], bf, name=f"attn{h}") for h in range(QH)]

        # ---------------- Phase A: projections + RoPE + V transpose --------
        with tc.tile_pool(name="accp", bufs=1, space="PSUM") as accp, \
             tc.tile_pool(name="ropep", bufs=1, space="PSUM") as ropep, \
             tc.tile_pool(name="vtp", bufs=1, space="PSUM") as vtp, \
             tc.tile_pool(name="stage", bufs=2) as stage:
            for j in range(J):
                sq = slice(j * 512, (j + 1) * 512)
                accs = [accp.tile([128, 512], f32, name=f"acc{m}") for m in range(6)]
                for g in range(G):
                    if (j, g) in xpre:
                        xt = xpre.pop((j, g))
                    else:
                        xt = xgp.tile([128, GD * 512], bf, tag="xt")
                        nc.sync.dma_start(xt[:], xp[j, g])
                        if j == 0 and g == G - 1:
                            emit_const_dmas()
                    for dd in range(GD):
                        d = g * GD + dd
                        xs = xt[:, dd * 512:(dd + 1) * 512]
                        st = (d == 0)
                        sp_ = (d == DCH - 1)
                        qb = d * 512
                        for m in range(QH):
                            nc.tensor.matmul(
                                accs[m][:], wq_sb[:, qb + m * 128:qb + (m + 1) * 128],
                                xs, start=st, stop=sp_)
                        kb = d * 256
                        nc.tensor.matmul(accs[4][:], wkv_sb[:, kb:kb + 128],
                                         xs, start=st, stop=sp_)
                        nc.tensor.matmul(accs[5][:], wkv_sb[:, kb + 128:kb + 256],
                                         xs, start=st, stop=sp_)

                # RoPE for q tiles and k tile
                for m in range(5):
                    dst = qrot[m][:, sq] if m < QH else krot[:, sq]
                    q_sb = stage.tile([128, 512], bf, tag="q_sb")
                    nc.scalar.copy(q_sb[:], accs[m][:])
                    rot_ps = ropep.tile([128, 512], f32, tag="rope_ps")
                    nc.tensor.matmul(rot_ps[:], rmat[:], q_sb[:],
                                     start=True, stop=True)
                    t1 = stage.tile([128, 512], f32, tag="t1")
                    nc.vector.tensor_mul(t1[:], accs[m][:], cosT[:, sq])
                    t2 = stage.tile([128, 512], f32, tag="t2")
                    nc.vector.tensor_mul(t2[:], rot_ps[:], sinT[:, sq])
                    nc.vector.tensor_add(dst, t1[:], t2[:])
                # region-scaled K columns (keys < 640 only)
                if j == 0:
                    nc.vector.tensor_mul(krot_s[:, 0:512], krot[:, 0:512],
                                         ksB[:, 0:512])
                elif j == 1:
                    nc.vector.tensor_mul(krot_s[:, 512:640], krot[:, 512:640],
                                         ksB[:, 512:640])

                # V: copy to SBUF, transpose 128x128 blocks into vnat
                v_sb = stage.tile([128, 512], bf, tag="v_sb")
                nc.scalar.copy(v_sb[:], accs[5][:])
                vt_ps = vtp.tile([128, 512], bf, tag="vt_ps")
                for b in range(4):
                    nc.tensor.transpose(vt_ps[:, b * 128:(b + 1) * 128],
                                        v_sb[:, b * 128:(b + 1) * 128], ident[:])
                nc.vector.tensor_copy(
                    vnat[:, (4 * j) * 128:(4 * j + 4) * 128], vt_ps[:])

        # ------------- Phase B+C: attention fused with o_proj --------------
        # Software-pipelined emission: the PE queue is in-order, so the
        # scores matmul of tile t+1 is EMITTED before the dn/av matmuls of
        # tile t (which wait on exp(t) on ACT). o_proj chunks of sq-block
        # j-1 (pure PE work, no ACT dependency) are interleaved into the
        # attention stream of block j so PE never idles on the exp/mask
        # chain. Mostly-masked diagonal tiles are trimmed to their live
        # column range (delta*128..512). Finalize (normalize) of a head is
        # emitted one tile into the next head; the [1,512]->[128,512]
        # reciprocal broadcast rides a DRAM round-trip DMA (no PE/PSUM).
        with tc.tile_pool(name="att_sb", bufs=4) as att_sb, \
             tc.tile_pool(name="sp", bufs=2, space="PSUM") as sp, \
             tc.tile_pool(name="avp", bufs=2, space="PSUM") as avp, \
             tc.tile_pool(name="dnp", bufs=2, space="PSUM") as dnp, \
             tc.tile_pool(name="op", bufs=2, space="PSUM") as op, \
             tc.tile_pool(name="nrm", bufs=3) as nrm, \
             tc.tile_pool(name="ost", bufs=2) as ost:
            # interleave plan: B items of block j + C chunks of block j-1
            plan = []
            # j=3 processes head 0 last so the trailing o_proj chunks (whose
            # matmuls end with head 0) wait minimally on its finalize
            horder = {j: list(range(QH)) for j in range(J)}
            horder[J - 1] = [1, 2, 3, 0]
            for j in range(J):
                bi = [("B", j, h, i)
                      for h in horder[j] for i in range(4 * j + 4)]
                cc = [("C", j - 1, c) for c in range(DCH)] if j > 0 else []
                merged = []
                nb, ncc = len(bi), len(cc)
                ci = 0
                for idx, el in enumerate(bi):
                    merged.append(el)
                    # hold C back a few items so attn[3] finalize lands
                    while (ci < ncc
                           and idx >= 3
                           and ci + 1 <= (idx - 2) * ncc / (nb - 3)):
                        merged.append(cc[ci])
                        ci += 1
                merged.extend(cc[ci:])
                plan.extend(merged)
            plan.extend(("C", J - 1, c) for c in range(DCH))
            b_seq = [el for el in plan if el[0] == "B"]
            b_next = {}
            for a, b in zip(b_seq, b_seq[1:]):
                b_next[a] = b

            def emit_scores(el):
                _, j, h, i = el
                delta = i - 4 * j
                c0 = 128 * delta if delta >= 1 else 0   # trimmed live cols
                sq0 = j * 512
                s_ps = sp.tile([128, 512], f32, tag="s")
                # scaled K for full-region rows (j>=2), keys<640
                if j >= 2 and i < 5:
                    kt = krot_s[:, i * 128:(i + 1) * 128]
                else:
                    kt = krot[:, i * 128:(i + 1) * 128]
                nc.tensor.matmul(s_ps[:, c0:512], kt,
                                 qrot[h][:, sq0 + c0:sq0 + 512],
                                 start=True, stop=True)
                if j == 1 and i < 5:
                    # partial region rows: queries 611.. are cols 99..
                    nc.vector.tensor_scalar_mul(
                        s_ps[:, 99:512], s_ps[:, 99:512], ksT[:, i:i + 1])
                return s_ps

            state = {}

            def emit_tail(el, s_ps):
                _, j, h, i = el
                ni = 4 * j + 4
                delta = i - 4 * j
                c0 = 128 * delta if delta >= 1 else 0
                if i == 0:
                    state[(j, h)] = (
                        avp.tile([128, 512], f32, tag="av", name="acc_av"),
                        dnp.tile([1, 512], f32, tag="dn", name="acc_dn"))
                acc_av, acc_dn = state[(j, h)]
                e_sb = att_sb.tile([128, 512], bf, tag="e")
                nc.scalar.activation(e_sb[:, c0:512], s_ps[:, c0:512], EXP)
                if delta >= 0:
                    nc.vector.tensor_mul(
                        e_sb[:, c0:512], e_sb[:, c0:512],
                        masks[:, delta * 512 + c0:(delta + 1) * 512])
                st = (i == 0)
                sp_l = (i == ni - 1)
                eb = e_sb[:, c0:512]
                nc.tensor.matmul(acc_dn[:, c0:512], ones_col[:], eb,
                                 start=st, stop=sp_l)
                nc.tensor.matmul(acc_av[:, c0:512],
                                 vnat[:, i * 128:(i + 1) * 128],
                                 eb, start=st, stop=sp_l)

            def emit_finalize(j, h):
                sq = slice(j * 512, (j + 1) * 512)
                acc_av, acc_dn = state.pop((j, h))
                # 1/x = exp(-ln(x)) on ACT; partition-broadcast [1,512] ->
                # [128,512] via a DRAM round-trip (0-stride DRAM source AP)
                # so no PE matmul / PSUM bank is needed.
                lrec = nrm.tile([1, 512], f32, tag="lrec")
                nc.scalar.activation(lrec[:], acc_dn[:],
                                     mybir.ActivationFunctionType.Ln)
                rec2 = nrm.tile([1, 512], f32, tag="rec2")
                nc.scalar.activation(rec2[:], lrec[:], EXP, scale=-1.0)
                hg = j * QH + h
                nc.sync.dma_start(rscr[hg:hg + 1, :], rec2[:])
                rb_sb = nrm.tile([128, 512], f32, tag="rb_sb")
                nc.sync.dma_start(rb_sb[:],
                                  rscr[hg:hg + 1, :].broadcast_to((128, 512)))
                nc.vector.tensor_mul(attn[h][:, sq], acc_av[:], rb_sb[:])

            osb_state = {}

            def emit_cchunk(jc, c):
                sq = slice(jc * 512, (jc + 1) * 512)
                if c % 4 == 0:
                    osb_state[jc] = ost.tile([128, 4 * 512], bf, tag="o_sb",
                                             name="o_sb")
                o_sb = osb_state[jc]
                cb = c % 4
                o_ps = op.tile([128, 512], f32, tag="o")
                for hi, h in enumerate([1, 2, 3, 0]):
                    nc.tensor.matmul(
                        o_ps[:],
                        wo_sb[:, h * D + c * 128:h * D + (c + 1) * 128],
                        attn[h][:, sq], start=(hi == 0), stop=(hi == QH - 1))
                # evictions stay off ACT: the strict-FIFO ACT queue would
                # delay the exps that pace the attention pipeline
                nc.vector.tensor_copy(
                    o_sb[:, cb * 512:(cb + 1) * 512], o_ps[:])
                if cb == 3:
                    nc.sync.dma_start(out[jc, c // 4], o_sb[:])

            s2_cur = emit_scores(b_seq[0])
            pending = None
            for el in plan:
                if el[0] == "B":
                    nxt = b_next.get(el)
                    s2_next = emit_scores(nxt) if nxt is not None else None
                    emit_tail(el, s2_cur)
                    s2_cur = s2_next
                    if pending is not None and pending != (el[1], el[2]):
                        emit_finalize(*pending)
                        pending = None
                    if el[3] == 4 * el[1] + 3:    # last key tile of (j, h)
                        pending = (el[1], el[2])
                else:
                    # a C chunk reads all four attn heads of its block: any
                    # still-pending finalize for that block must land first
                    if pending is not None and pending[0] == el[1]:
                        emit_finalize(*pending)
                        pending = None
                    emit_cchunk(el[1], el[2])
            if pending is not None:
                emit_finalize(*pending)

    # Split multi-wait instructions onto standalone EventSemaphore insts.
    import bass_rust
    bass_rust.generate_event_semaphores(nc)
    return nc


def _get_compiled():
    if "nc" not in _CACHE:
        _CACHE["nc"] = _build_bass()
        _CACHE["const"] = _host_constants()
    return _CACHE["nc"], _CACHE["const"]


def kernel(hidden_states, wq, wk, wv, wo, _trace=False):
    from concourse.bass_utils import run_bass_kernel_spmd

    nc, cst = _get_compiled()
    bf16 = _bf16()

    x = np.asarray(hidden_states, dtype=np.float32).reshape(S, D)
    xT = x.T                                             # [D, S]
    # xp[j, g, p, dd*512+f] = xT[128*(8g+dd)+p, 512j+f]
    t = xT.reshape(G, GD, 128, J, 512)                   # [g, dd, p, j, f]
    xpk = np.ascontiguousarray(
        t.transpose(3, 0, 2, 1, 4).reshape(J, G, 128, GD * 512)).astype(bf16)
    wq = np.asarray(wq, dtype=np.float32)
    wk = np.asarray(wk, dtype=np.float32)
    wv = np.asarray(wv, dtype=np.float32)
    wo = np.asarray(wo, dtype=np.float32)
    scale = 1.0 / math.sqrt(HD)

    in_maps = []
    for d in range(NCORES):
        wq_d = wq[d * QH * HD:(d + 1) * QH * HD] * scale      # [512, D]
        wqT = wq_d.T                                          # [4096, 512]
        wqpk = np.ascontiguousarray(
            wqT.reshape(G, GD, 128, 512).transpose(0, 2, 1, 3)
            .reshape(G, 128, GD * 512)).astype(bf16)
        wkT = wk[d * HD:(d + 1) * HD].T.reshape(G, GD, 128, 128)
        wvT = wv[d * HD:(d + 1) * HD].T.reshape(G, GD, 128, 128)
        kv = np.concatenate([wkT, wvT], axis=-1)              # [G, GD, 128, 256]
        wkvpk = np.ascontiguousarray(
            kv.transpose(0, 2, 1, 3).reshape(G, 128, GD * 256)).astype(bf16)
        woT = wo[:, d * QH * HD:(d + 1) * QH * HD].T          # [512, 4096]
        wopk = np.ascontiguousarray(
            woT.reshape(QH, 128, D).transpose(1, 0, 2)
            .reshape(128, QH * D)).astype(bf16)
        in_maps.append({
            "xp": xpk,
            "wqp": wqpk,
            "wkvp": wkvpk,
            "wop": wopk,
            "cosT": cst["cosT"], "sinT": cst["sinT"],
            "rmat": cst["rmat"], "ident": cst["ident"],
            "masks": cst["masks"], "ksT": cst["ksT"], "ksB": cst["ksB"],
            "ones_col": cst["ones_col"],
        })

    res = run_bass_kernel_spmd(nc, in_maps, core_ids=list(range(NCORES)),
                               trace=_trace)
    acc = res.results[0]["out"].astype(np.float32)
    for d in range(1, NCORES):
        acc += res.results[d]["out"].astype(np.float32)
    # out_t[j, cq, p, cb*512+f] = out.T[128*(4cq+cb)+p, 512j+f]
    acc = acc.reshape(J, DCH // 4, 128, 4, 512)          # [j, cq, p, cb, f]
    outp = acc.transpose(0, 4, 1, 3, 2).reshape(S, D)    # [512j+f, 128(4cq+cb)+p]
    outp = outp.reshape(1, S, D).astype(np.float32)
    if _trace:
        _CACHE["last_results"] = res
    return outp
